# revision 1
# baseline (speedup 1.0000x reference)
"""DAML dense_cnn Trainium2 Bass kernel.

Data-parallel over batch: B=64 -> 8 NeuronCores x 8 batches each.

Per-core pipeline (per side u/i):
  1. dma_gather (transpose mode, bf16) pulls e^T = emb[doc]^T directly into
     SBUF as (128 dpart, 3 chunks, 1024 tok-cols) per 2-batch group. The
     50k-vocab int16-index limit is beaten by biasing the table base by
     32768 rows and feeding idx-32768 (the ucode sign-extends, so negative
     offsets address the low half). Pad/guard positions gather a
     host-appended all-zero row (id 50000), giving exact zero conv padding.
  2. gate: 3 matmuls (K=128 chunks, M=3 taps, N=502) -> psum u(3,502);
     2 DVE adds combine the taps with column shifts; ACT Sigmoid(+wcb bias)
     -> g (1,500); gpsimd partition_broadcast -> (128,500); DVE mult gates
     e^T in place.
  3. conv: 9 matmuls (3 taps x 3 chunks) accumulate feat psum (100,500)
     via shifted rhs windows.  feat -> sbuf (102,500) bf16 (+dc_b bias,
     user side scaled by -2).  Aug rows: row {100|101} = sum_f feat^2
     (user x0.25), other = ones.  K=102 einsum then yields
     sq[l,m] = |u_l - v_m|^2 directly in psum.
  4. att = sigmoid(-0.5*ln(sq)) == 1/(1+sqrt(sq)).  ACT Ln then ACT Sigmoid
     with accum_out giving user row-sums free; item col-sums via
     ones-matmuls on att tiles.  Chunked so ACT table loads stay rare.
  5. Pooling collapses to S_k[f] = sum_l c_k[l]*att_sum[l]*feat[f,l]
     (c_k = sliding-window counts): PE transposes of feat + small matmuls,
     abs-conv contraction (bias via aug row), fc matmul, ACT relu(+bias),
     PE transpose, id-embedding indirect gather, DMA out.
"""
import os
import numpy as np
import ml_dtypes

import concourse.bass as bass
import concourse.bacc as bacc
import concourse.tile as tile
from concourse import mybir
from concourse import bass_utils

BF16 = ml_dtypes.bfloat16
DT = mybir.dt
AF = mybir.ActivationFunctionType
ALU = mybir.AluOpType

B, L, V, D, F, ID = 64, 500, 50000, 300, 100, 32
NCORE = 8
BLOC = B // NCORE            # batches per core
DPAD = 384                   # D padded to 3*128
NCH = 3
PADROW = V                   # zero row appended to tables
BIAS = 32768                 # int16 index bias
GW = 512                     # tokens per gather group (1 batch)
NG = BLOC                    # gather groups per side
SEG = 502                    # batch segment stride inside a group
LT = [128, 128, 128, 116]
LT_OFF = [0, 128, 256, 384]
ATT_CHUNK = int(os.environ.get("DAML_ATT_CHUNK", "8"))
STAGE = int(os.environ.get("DAML_STAGE", "4"))
NO_ACCUM = os.environ.get("DAML_NO_ACCUM", "1") == "1"
NO_IATT = os.environ.get("DAML_NO_IATT", "0") == "1"
NO_LNBIAS = os.environ.get("DAML_NO_LNBIAS", "0") == "1"
LOOP = int(os.environ.get("DAML_LOOP", "1"))
GATE_CT = os.environ.get("DAML_GATE_CT", "0") == "1"


def build_program():
    nc = bacc.Bacc("TRN2", target_bir_lowering=False, debug=False,
                   num_devices=NCORE)
    t = {}

    def din(name, shape, dt):
        t[name] = nc.dram_tensor(name, shape, dt, kind="ExternalInput")

    for s in "ui":
        din(f"tab_{s}", (V + 1, DPAD), DT.bfloat16)
        din(f"idx_{s}", (128, 32 * NG), DT.int16)
        din(f"wd_{s}", (128, NCH, 3, F), DT.bfloat16)
        din(f"dcb_{s}", (F, 1), DT.float32)
        din(f"wabs_{s}", (101, 3, F), DT.bfloat16)
        din(f"wfc_{s}", (F, ID), DT.bfloat16)
        din(f"bfc_{s}", (ID, 1), DT.float32)
        din(f"idemb_{s}", (10002, ID), DT.float32)
        din(f"idids_{s}", (BLOC, 1), DT.int32)
    din("w3", (128, NCH, 3), DT.bfloat16)
    din("wcb", (1, 1), DT.float32)
    din("ck", (128, 4, 3), DT.bfloat16)
    din("ones_row", (1, L), DT.bfloat16)
    t["v2s_scratch"] = nc.dram_tensor("v2s_scratch", (BLOC, L), DT.bfloat16,
                                      kind="Internal")
    t["dbg"] = nc.dram_tensor("dbg", (128, 3 * GW), DT.float32,
                              kind="ExternalOutput")
    t["out_use"] = nc.dram_tensor("out_use", (BLOC, 2 * ID), DT.float32,
                                  kind="ExternalOutput")
    t["out_item"] = nc.dram_tensor("out_item", (BLOC, 2 * ID), DT.float32,
                                   kind="ExternalOutput")

    with tile.TileContext(nc) as tc:
        _emit(nc, tc, t)

    nc.compile()
    return nc


def _emit(nc, tc, t):
    from contextlib import ExitStack
    from concourse.masks import make_identity
    ctx = ExitStack()

    consts = ctx.enter_context(tc.tile_pool(name="consts", bufs=1))
    et_pool = ctx.enter_context(tc.tile_pool(name="et", bufs=1))
    feat_pool = ctx.enter_context(tc.tile_pool(name="feat", bufs=1))
    ln_pool = ctx.enter_context(tc.tile_pool(name="ln", bufs=2))
    att_pool = ctx.enter_context(tc.tile_pool(name="att", bufs=2))
    sm_pool = ctx.enter_context(tc.tile_pool(name="sm", bufs=4))
    gbc_pool = ctx.enter_context(tc.tile_pool(name="gbc", bufs=1))
    u2_pool = ctx.enter_context(tc.tile_pool(name="u2", bufs=2))
    ft_pool = ctx.enter_context(tc.tile_pool(name="ft", bufs=2))

    psum = ctx.enter_context(tc.tile_pool(name="psum", bufs=2, space="PSUM"))
    psg = ctx.enter_context(tc.tile_pool(name="psg", bufs=1, space="PSUM"))
    ps_S = ctx.enter_context(tc.tile_pool(name="ps_S", bufs=1, space="PSUM"))

    # ---------------- constants / weights ----------------
    # order matters: idx + gate weights first so gathers + gate start early
    idx_sb, wd_sb, wabs_sb, wfc_sb, bfc_sb, dcb_sb, idid_sb = ({} for _ in range(7))
    for s in "ui":
        idx_sb[s] = consts.tile([128, 32 * NG], DT.int16, tag=f"idx{s}", name=f"idx{s}")
        nc.sync.dma_start(out=idx_sb[s][:], in_=t[f"idx_{s}"].ap())
    w3_sb = consts.tile([128, NCH, 3], DT.bfloat16, tag="w3", name="w3")
    nc.sync.dma_start(out=w3_sb[:], in_=t["w3"].ap())
    wcb_sb = consts.tile([1, 1], DT.float32, tag="wcb", name="wcb")
    nc.sync.dma_start(out=wcb_sb[:], in_=t["wcb"].ap())
    wcb_bc = consts.tile([128, 1], DT.float32, tag="wcbb", name="wcbb")
    nc.gpsimd.partition_broadcast(wcb_bc[:, :], wcb_sb[:, :])
    for s in "ui":
        wd_sb[s] = consts.tile([128, NCH, 3, F], DT.bfloat16, tag=f"wd{s}", name=f"wd{s}")
        nc.sync.dma_start(out=wd_sb[s][:], in_=t[f"wd_{s}"].ap())
        dcb_sb[s] = consts.tile([F, 1], DT.float32, tag=f"dcb{s}", name=f"dcb{s}")
        nc.sync.dma_start(out=dcb_sb[s][:], in_=t[f"dcb_{s}"].ap())
    for s in "ui":
        wabs_sb[s] = consts.tile([101, 3, F], DT.bfloat16, tag=f"wabs{s}", name=f"wabs{s}")
        nc.sync.dma_start(out=wabs_sb[s][:], in_=t[f"wabs_{s}"].ap())
        wfc_sb[s] = consts.tile([F, ID], DT.bfloat16, tag=f"wfc{s}", name=f"wfc{s}")
        nc.sync.dma_start(out=wfc_sb[s][:], in_=t[f"wfc_{s}"].ap())
        bfc_sb[s] = consts.tile([ID, 1], DT.float32, tag=f"bfc{s}", name=f"bfc{s}")
        nc.sync.dma_start(out=bfc_sb[s][:], in_=t[f"bfc_{s}"].ap())
        idid_sb[s] = consts.tile([BLOC, 1], DT.int32, tag=f"idid{s}", name=f"idid{s}")
        nc.sync.dma_start(out=idid_sb[s][:], in_=t[f"idids_{s}"].ap())
    ck_sb = consts.tile([128, 4, 3], DT.bfloat16, tag="ck", name="ck")
    nc.sync.dma_start(out=ck_sb[:], in_=t["ck"].ap())

    ones_bf = consts.tile([128, 1], DT.bfloat16, tag="ones", name="ones")
    nc.vector.memset(ones_bf[:], 1.0)
    ident_bf = consts.tile([128, 128], DT.bfloat16, tag="identb", name="identb")
    make_identity(nc, ident_bf[:])
    ident_f32 = consts.tile([ID, ID], DT.float32, tag="identf", name="identf")
    make_identity(nc, ident_f32[:])

    if LOOP > 1:
        ctx.enter_context(tc.For_i(0, LOOP, 1))

    # ---------------- gathers (per 2-batch group) ----------------
    eT = {}
    for g in range(NG):
        for s in "ui":
            eT[(s, g)] = et_pool.tile([128, NCH, GW], DT.bfloat16, tag=f"eT{s}{g}", name=f"eT{s}{g}")
            nc.gpsimd.dma_gather(
                out_ap=eT[(s, g)][:],
                in_ap=t[f"tab_{s}"].ap()[BIAS:, :],
                idxs_ap=idx_sb[s][:, 32 * g:32 * (g + 1)],
                num_idxs=GW, num_idxs_reg=GW,
                elem_size=DPAD, transpose=True,
            )

    feat_sb = {(s, b): feat_pool.tile([101, L], DT.bfloat16, tag=f"feat{s}{b}", name=f"feat{s}{b}")
               for s in "ui" for b in range(BLOC)}
    uattT = consts.tile([128, 4, BLOC], DT.float32, tag="uatt", name="uatt")
    iattT = {}

    # ---------------- per-batch stage blocks ----------------
    g_bc = {}
    u2sT_b = {}

    GATE_PACK = os.environ.get("DAML_GATE_PACK", "0") == "1"

    def gate_pair(b0):
        # gates for (u,b0),(i,b0),(u,b0+1),(i,b0+1) packed at psum
        # partitions 0/32/64/96 so ONE sigmoid covers all four rows.
        quads = [("u", b0), ("i", b0), ("u", b0 + 1), ("i", b0 + 1)]
        if not GATE_PACK:
            for s, g in quads:
                ups = psg.tile([1, L], DT.float32, tag="gate", name="gate")
                k = 0
                for tp in range(3):
                    for c in range(NCH):
                        nc.tensor.matmul(out=ups[:, :],
                                         lhsT=w3_sb[:, c, tp:tp + 1],
                                         rhs=eT[(s, g)][:, c, tp:tp + L],
                                         start=(k == 0), stop=(k == 8))
                        k += 1
                g_sb = sm_pool.tile([1, L], DT.bfloat16, tag="g4", name="g4")
                nc.scalar.activation(g_sb[:, :], ups[:, :], AF.Sigmoid,
                                     bias=wcb_sb[:, :])
                gb = gbc_pool.tile([128, L], DT.bfloat16, tag=f"g_bc{s}{g}", name="g_bc")
                nc.gpsimd.partition_broadcast(gb[:, :], g_sb[:, :])
                g_bc[(s, g)] = gb
            return
        ups = psg.tile([97, L], DT.float32, tag="gate", name="gate")
        for qi, (s, g) in enumerate(quads):
            k = 0
            for tp in range(3):
                for c in range(NCH):
                    nc.tensor.matmul(out=ups[32 * qi:32 * qi + 1, :],
                                     lhsT=w3_sb[:, c, tp:tp + 1],
                                     rhs=eT[(s, g)][:, c, tp:tp + L],
                                     start=(k == 0), stop=(k == 8),
                                     tile_position=(0, 32 * qi))
                    k += 1
        g4 = sm_pool.tile([97, L], DT.bfloat16, tag="g4", name="g4")
        nc.scalar.activation(g4[:, :], ups[:, :], AF.Sigmoid,
                             bias=wcb_bc[0:97, :])
        for qi, (s, g) in enumerate(quads):
            gb = gbc_pool.tile([128, L], DT.bfloat16, tag=f"g_bc{s}{g}", name="g_bc")
            nc.gpsimd.partition_broadcast(gb[:, :], g4[32 * qi:32 * qi + 1, :])
            g_bc[(s, g)] = gb

    def conv_block(s, b):
        g, base = b, 0
        tok = base + 1
        gb = g_bc[(s, b)]
        nc.vector.tensor_tensor(out=eT[(s, g)][:, :, tok:tok + L],
                                in0=eT[(s, g)][:, :, tok:tok + L],
                                in1=gb[:, None, :].to_broadcast([128, NCH, L]),
                                op=ALU.mult)
        fps = psum.tile([F, L], DT.float32, tag="feat", name="feat")
        k = 0
        for tp in range(3):
            for c in range(NCH):
                nc.tensor.matmul(out=fps[:, :], lhsT=wd_sb[s][:, c, tp, :],
                                 rhs=eT[(s, g)][:, c, base + tp:base + tp + L],
                                 start=(k == 0), stop=(k == 8))
                k += 1
        fsb = feat_sb[(s, b)]
        nc.vector.tensor_scalar(out=fsb[0:F, :], in0=fps[:, :],
                                scalar1=(-2.0 if s == "u" else 1.0),
                                scalar2=dcb_sb[s][:, :],
                                op0=ALU.mult, op1=ALU.add)
        u2 = u2_pool.tile([F, L], DT.bfloat16, tag="u2", name="u2")
        nc.vector.tensor_tensor(out=u2[:, :], in0=fsb[0:F, :],
                                in1=fsb[0:F, :], op=ALU.mult)
        if s == "u":
            # row 100 = ones (DMA: engine writes must be 32-aligned)
            nc.sync.dma_start(out=fsb[100:101, :], in_=t["ones_row"].ap())
            # u2sT[l] = 0.25 * sum_f (-2 feat)^2, l on partitions (ln bias)
            ut = psum.tile([128, 4], DT.float32, tag="u", name="ut")
            for lt in range(4):
                m = LT[lt]
                nc.tensor.matmul(out=ut[0:m, lt:lt + 1],
                                 lhsT=u2[:, LT_OFF[lt]:LT_OFF[lt] + m],
                                 rhs=ones_bf[0:F, :], start=True, stop=True)
            u2sT = consts.tile([128, 4], DT.float32, tag=f"u2sT{b}",
                               name=f"u2sT{b}")
            nc.vector.tensor_scalar(out=u2sT[:, :], in0=ut[:, :],
                                    scalar1=0.25, scalar2=None, op0=ALU.mult)
            u2sT_b[b] = u2sT
        else:
            # row 100 = v2s = sum_f feat^2 (stage via partition 0 + DMA)
            v2s = psum.tile([1, L], DT.float32, tag="u", name="v2s")
            nc.tensor.matmul(out=v2s[:, :], lhsT=ones_bf[0:F, :], rhs=u2[:, :],
                             start=True, stop=True)
            v2st = sm_pool.tile([1, L], DT.bfloat16, tag="v2st", name="v2st")
            nc.vector.tensor_copy(v2st[:, :], v2s[:, :])
            # bounce via DRAM: SBUF->SBUF DMA deadlocks vs xbar gathers
            nc.sync.dma_start(out=t["v2s_scratch"].ap()[b:b + 1, :],
                              in_=v2st[:, :])
            nc.sync.dma_start(out=fsb[100:101, :],
                              in_=t["v2s_scratch"].ap()[b:b + 1, :])

    ln_live = {}

    def sq_ln_block(b):
        ln_tiles = {}
        for lt in range(4):
            m = LT[lt]
            sq = psum.tile([128, L], DT.float32, tag="sq", name="sq")
            nc.tensor.matmul(
                out=sq[:m, :],
                lhsT=feat_sb[("u", b)][:, LT_OFF[lt]:LT_OFF[lt] + m],
                rhs=feat_sb[("i", b)][:, :],
                start=True, stop=True)
            lnt = ln_pool.tile([128, L], DT.bfloat16, tag=f"lnt{lt}", name="lnt")
            if NO_LNBIAS:
                nc.scalar.activation(lnt[:m, :], sq[:m, :], AF.Ln)
            else:
                nc.scalar.activation(lnt[:m, :], sq[:m, :], AF.Ln,
                                     bias=u2sT_b[b][0:m, lt:lt + 1])
            ln_tiles[lt] = lnt
        ln_live[b] = ln_tiles

    att_live = {}

    def sig_block(b):
        ln_tiles = ln_live.pop(b)
        atts = {}
        for lt in range(4):
            m = LT[lt]
            att = att_pool.tile([128, L], DT.bfloat16, tag=f"att{lt}", name="att")
            atts[lt] = att
            if NO_ACCUM:
                nc.scalar.activation(att[:m, :], ln_tiles[lt][:m, :],
                                     AF.Sigmoid, scale=-0.5)
                nc.vector.tensor_reduce(out=uattT[:m, lt, b:b + 1],
                                        in_=att[:m, :],
                                        axis=mybir.AxisListType.X,
                                        op=ALU.add)
            else:
                nc.scalar.activation(att[:m, :], ln_tiles[lt][:m, :],
                                     AF.Sigmoid, scale=-0.5,
                                     accum_out=uattT[:m, lt, b:b + 1])
        att_live[b] = atts

    def iatt_block(b):
        atts = att_live.pop(b)
        ia = psum.tile([128, 4], DT.float32, tag="feat", name="ia")
        # column sums: 4 accumulating K-matmuls per ms-tile, directly on the
        # att row-tiles (no DVE pre-sum).
        for ms in range(4):
            if NO_IATT:
                break
            for lt in range(4):
                nc.tensor.matmul(
                    out=ia[0:LT[ms], ms:ms + 1],
                    lhsT=atts[lt][:LT[lt], LT_OFF[ms]:LT_OFF[ms] + LT[ms]],
                    rhs=ones_bf[:LT[lt], :],
                    start=(lt == 0), stop=(lt == 3))
        iab = consts.tile([128, 4], DT.float32, tag=f"iatt{b}", name=f"iatt{b}")
        nc.vector.tensor_copy(iab[:, :], ia[:, :])
        iattT[b] = iab

    S_ps = ps_S.tile([F, 6 * BLOC], DT.float32, tag="S", name="S")

    def pool_block(s, b):
        si = 0 if s == "u" else 1
        col = 3 * (BLOC * si + b)
        wa = sm_pool.tile([128, 4, 3], DT.bfloat16, tag="wa", name="wa")
        attsrc = uattT[:, :, b:b + 1] if s == "u" else iattT[b][:, :, None]
        nc.vector.tensor_tensor(out=wa[:, :, :],
                                in0=attsrc.to_broadcast([128, 4, 3]),
                                in1=ck_sb[:, :, :], op=ALU.mult)
        # 4 transposes into one psum tile (sequential col groups), 1 copy
        ftp = psum.tile([128, 4 * F], DT.bfloat16, tag="sq", name="ftp")
        for lt in range(4):
            m = LT[lt]
            nc.tensor.transpose(
                ftp[:m, F * lt:F * (lt + 1)],
                feat_sb[(s, b)][0:F, LT_OFF[lt]:LT_OFF[lt] + m],
                ident_bf[0:F, 0:F])
        fts = ft_pool.tile([128, 4 * F], DT.bfloat16, tag="fts", name="fts")
        nc.vector.tensor_copy(fts[:, :], ftp[:, :])
        for lt in range(4):
            m = LT[lt]
            nc.tensor.matmul(out=S_ps[:, col:col + 3],
                             lhsT=fts[:m, F * lt:F * (lt + 1)],
                             rhs=wa[:m, lt, :], start=(lt == 0), stop=(lt == 3))

    # ---------------- pipelined driver ----------------
    # ACT table sets hold either Ln or Sigmoid, never both (1283ns reload
    # per switch), so ACT work is emitted in alternating contiguous blocks:
    # [sigG0][Ln01][sigG1 Sig01][Ln23][sigG2 Sig23]... while PE gets a
    # dense interleave of gate/conv/sq/iatt matmuls.  iatt (PE) is split
    # from sig (ACT) so PE's 4-deep wait queue never blocks on ACT.
    gate_pair(0)
    gate_pair(2)
    gate_pair(4)
    gate_pair(6)
    for b in (0, 1):
        conv_block("u", b)
        conv_block("i", b)
    sq_ln_block(0)
    sq_ln_block(1)
    for b in (2, 3):
        conv_block("u", b)
        conv_block("i", b)
    sig_block(0)
    sig_block(1)
    iatt_block(0)
    iatt_block(1)
    for b in (4, 5):
        conv_block("u", b)
        conv_block("i", b)
    sq_ln_block(2)
    sq_ln_block(3)
    pool_block("u", 0), pool_block("i", 0)
    pool_block("u", 1), pool_block("i", 1)
    sig_block(2)
    sig_block(3)
    iatt_block(2)
    iatt_block(3)
    for b in (6, 7):
        conv_block("u", b)
        conv_block("i", b)
    sq_ln_block(4)
    sq_ln_block(5)
    pool_block("u", 2), pool_block("i", 2)
    pool_block("u", 3), pool_block("i", 3)
    sig_block(4)
    sig_block(5)
    iatt_block(4)
    iatt_block(5)
    sq_ln_block(6)
    sq_ln_block(7)
    sig_block(6)
    sig_block(7)
    iatt_block(6)
    iatt_block(7)
    for bb in (4, 5, 6, 7):
        pool_block("u", bb)
        pool_block("i", bb)

    S_sb = consts.tile([101, 6 * BLOC], DT.bfloat16, tag="Ssb", name="Ssb")
    nc.vector.tensor_copy(S_sb[0:F, :], S_ps[:, :])
    nc.sync.dma_start(out=S_sb[F:F + 1, :], in_=t["ones_row"].ap()[:, 0:6 * BLOC])

    am_ps = psum.tile([F, 2 * BLOC], DT.float32, tag="feat", name="feat")
    for si, s in enumerate("ui"):
        for b in range(BLOC):
            for k in range(3):
                nc.tensor.matmul(
                    out=am_ps[:, BLOC * si + b:BLOC * si + b + 1],
                    lhsT=wabs_sb[s][:, k, :],
                    rhs=S_sb[:, 3 * (BLOC * si + b) + k:3 * (BLOC * si + b) + k + 1],
                    start=(k == 0), stop=(k == 2))
    am_sb = sm_pool.tile([F, 2 * BLOC], DT.bfloat16, tag="am_sb", name="am_sb")
    nc.vector.tensor_copy(am_sb[:, :], am_ps[:, :])

    for si, (s, oname) in enumerate((("u", "out_use"), ("i", "out_item"))):
        fc_ps = psum.tile([ID, BLOC], DT.float32, tag="u", name="u")
        nc.tensor.matmul(out=fc_ps[:, :], lhsT=wfc_sb[s][:, :],
                         rhs=am_sb[:, BLOC * si:BLOC * (si + 1)],
                         start=True, stop=True)
        fcr = sm_pool.tile([ID, BLOC], DT.float32, tag="fcr", name="fcr")
        nc.scalar.activation(fcr[:, :], fc_ps[:, :], AF.Relu,
                             bias=bfc_sb[s][:, :])
        fct = psum.tile([BLOC, ID], DT.float32, tag="u", name="u")
        nc.tensor.transpose(fct[:, :], fcr[:, :], ident_f32[:ID, :ID])
        osb = sm_pool.tile([BLOC, 2 * ID], DT.float32, tag=f"osb{s}", name=f"osb{s}")
        nc.vector.tensor_copy(osb[:, 0:ID], fct[:, :])
        nc.gpsimd.indirect_dma_start(
            out=osb[:, ID:2 * ID], out_offset=None,
            in_=t[f"idemb_{s}"].ap(),
            in_offset=bass.IndirectOffsetOnAxis(ap=idid_sb[s][:, 0:1], axis=0))
        nc.sync.dma_start(out=t[oname].ap(), in_=osb[:, :])

    ctx.close()


# ======================= host side =======================

_PROG = None


def _get_prog():
    global _PROG
    if _PROG is None:
        _PROG = build_program()
    return _PROG


def _bf16_table(tab):
    out = np.zeros((V + 1, DPAD), dtype=BF16)
    out[:V, :D] = np.asarray(tab, dtype=np.float32)
    return out


def _gather_idx(doc):
    """doc: (BLOC, L) ids -> (128, 32*NG) int16 biased index tile"""
    stream = np.full((NG, GW), PADROW, dtype=np.int64)
    for b in range(BLOC):
        stream[b, 1:1 + L] = doc[b]
    biased = (stream - BIAS).astype(np.int16)
    arr = np.zeros((128, 32 * NG), dtype=np.int16)
    for g in range(NG):
        blk = biased[g].reshape(32, 16).T  # idx i -> [i%16, i//16]
        for r in range(8):
            arr[16 * r:16 * (r + 1), 32 * g:32 * (g + 1)] = blk
    return arr


def _window_counts():
    c = np.zeros((3, L), dtype=np.float64)
    for k in range(3):
        for lp in range(k, k + L - 2):
            for d2 in (-1, 0, 1):
                ll = lp + d2
                if 0 <= ll < L:
                    c[k, ll] += 1
    return c


def _prep_weights(inp):
    w = {}
    w3 = np.zeros((DPAD, 3), dtype=np.float32)
    w3[:D, :] = np.asarray(inp["word_cnn_w"][0, 0]).astype(np.float32).T
    w["w3"] = np.ascontiguousarray(w3.reshape(NCH, 128, 3).transpose(1, 0, 2)).astype(BF16)
    w["wcb"] = np.asarray(inp["word_cnn_b"]).astype(np.float32).reshape(1, 1)

    for s, key in (("u", "user"), ("i", "item")):
        dw = np.asarray(inp[f"{key}_doc_cnn_w"]).astype(np.float32)  # (F,1,3,D)
        arr = np.zeros((128, NCH, 3, F), dtype=BF16)
        for tp in range(3):
            pad = np.zeros((DPAD, F), dtype=np.float32)
            pad[:D] = dw[:, 0, tp, :].T
            arr[:, :, tp, :] = pad.reshape(NCH, 128, F).transpose(1, 0, 2)
        w[f"wd_{s}"] = arr
        dcb = np.asarray(inp[f"{key}_doc_cnn_b"]).astype(np.float32)
        w[f"dcb_{s}"] = (dcb * (-2.0 if s == "u" else 1.0)).reshape(F, 1)

        aw = np.asarray(inp[f"{key}_abs_cnn_w"]).astype(np.float32)  # (F,1,3,F)
        ab = np.asarray(inp[f"{key}_abs_cnn_b"]).astype(np.float32)
        scale = (1.0 / (L - 2)) * (-0.5 if s == "u" else 1.0)
        warr = np.zeros((101, 3, F), dtype=BF16)
        for k in range(3):
            warr[:F, k, :] = (aw[:, 0, k, :] * scale).T
        warr[F, 0, :] = ab
        w[f"wabs_{s}"] = warr

        w[f"wfc_{s}"] = np.asarray(inp[f"{key}_fc_w"]).astype(np.float32).T.astype(BF16)
        w[f"bfc_{s}"] = np.asarray(inp[f"{key}_fc_b"]).astype(np.float32).reshape(ID, 1)

    cw = _window_counts()
    ckt = np.zeros((128, 4, 3), dtype=BF16)
    for lt in range(4):
        m = LT[lt]
        ckt[:m, lt, :] = cw[:, LT_OFF[lt]:LT_OFF[lt] + m].T
    w["ck"] = ckt
    return w


def prepare_in_maps(inputs):
    w = _prep_weights(inputs)
    tab_u = _bf16_table(inputs["user_word_emb"])
    tab_i = _bf16_table(inputs["item_word_emb"])
    user_doc = np.asarray(inputs["user_doc"]).astype(np.int64)
    item_doc = np.asarray(inputs["item_doc"]).astype(np.int64)
    uids = np.asarray(inputs["uids"]).astype(np.int64)
    iids = np.asarray(inputs["iids"]).astype(np.int64)
    uid_emb = np.asarray(inputs["uid_emb"]).astype(np.float32)
    iid_emb = np.asarray(inputs["iid_emb"]).astype(np.float32)

    in_maps = []
    for c in range(NCORE):
        sl = slice(BLOC * c, BLOC * (c + 1))
        in_maps.append({
            "tab_u": tab_u, "tab_i": tab_i,
            "idx_u": _gather_idx(user_doc[sl]),
            "idx_i": _gather_idx(item_doc[sl]),
            "w3": w["w3"], "wcb": w["wcb"], "ck": w["ck"],
            "ones_row": np.ones((1, L), dtype=BF16),
            "wd_u": w["wd_u"], "wd_i": w["wd_i"],
            "dcb_u": w["dcb_u"], "dcb_i": w["dcb_i"],
            "wabs_u": w["wabs_u"], "wabs_i": w["wabs_i"],
            "wfc_u": w["wfc_u"], "wfc_i": w["wfc_i"],
            "bfc_u": w["bfc_u"], "bfc_i": w["bfc_i"],
            # crossed on purpose: use_fea carries iid_emb, item_fea uid_emb
            "idemb_u": iid_emb, "idemb_i": uid_emb,
            "idids_u": iids[sl].astype(np.int32).reshape(BLOC, 1),
            "idids_i": uids[sl].astype(np.int32).reshape(BLOC, 1),
        })
    return in_maps


def assemble_outputs(res):
    use = np.concatenate([np.asarray(res.results[c]["out_use"]) for c in range(NCORE)])
    item = np.concatenate([np.asarray(res.results[c]["out_item"]) for c in range(NCORE)])
    return (use.reshape(B, 2, ID).astype(np.float32),
            item.reshape(B, 2, ID).astype(np.float32))


def kernel(**inputs):
    nc = _get_prog()
    in_maps = prepare_in_maps(inputs)
    res = bass_utils.run_bass_kernel_spmd(nc, in_maps, core_ids=list(range(NCORE)))
    return assemble_outputs(res)



# revision 30
# speedup vs baseline: 1.4509x; 1.4509x over previous
"""DAML dense_cnn Trainium2 Bass kernel (v2).

Data-parallel over batch: B=64 -> 8 NeuronCores x 8 batches each.

Per-core pipeline (per side u/i), restructured from v1 for engine balance:
  1. 16 dma_gathers (transpose, bf16, 512 idx each) pull e^T = emb[doc]^T
     into SBUF as (128 dpart, 3 chunks, 512 tok-cols) per batch-side,
     spread over 4 SWDGE queues. int16 index range beaten by biasing the
     table base by 32768 rows (ucode sign-extends). Pad positions gather a
     host-appended zero row.
  2. Gates in column form: per (side,batch) 36 tiny N=1 matmuls (tap
     shift on the lhsT eT slice) accumulate gate[l] on partitions, one
     column per 128-token block; Sigmoid in column form; 4 aligned PE
     transposes -> gate row; gpsimd partition_broadcast; DVE gating mult
     on eT.  PE stream time ~0 vs v1's 144 N=500 matmuls (~20us saved).
  3. conv: 9 matmuls (3 taps x 3 chunks) accumulate feat psum rows 2..101;
     aug rows computed in-psum (row0/1 = ones | sum_f feat^2 via
     K=100 ones-matmul writing psum row directly - no DRAM bounce).
     feat -> sbuf (102, 512) bf16, cols 500:512 zeroed so all sq matmuls
     run M=128.
  4. sq einsum K=102 now includes BOTH norm terms (aug rows), so Ln needs
     no bias -> Ln merged to (128,2,500) ops.  att = sigmoid(-0.5*ln(sq)),
     accum_out gives user row-sums free; item col-sums via N=1
     ones-matmuls (free on PE).
  5. ACT table thrash killed by wave scheduling (tc.tile_wait_until):
     [sig gates x4][Ln b0-3][sig b0-3][Ln b4-7][sig b4-7][relu] = 5 loads.
  6. Pooling: PE transposes of feat + S_k matmuls, abs-conv contraction
     (bias via aug row), fc matmul, ACT relu, PE transpose, id-emb
     indirect gather, DMA out.
"""
import os
import numpy as np
import ml_dtypes

import concourse.bass as bass
import concourse.bacc as bacc
import concourse.tile as tile
from concourse import mybir
from concourse import bass_utils

BF16 = ml_dtypes.bfloat16
DT = mybir.dt
AF = mybir.ActivationFunctionType
ALU = mybir.AluOpType

B, L, V, D, F, ID = 64, 500, 50000, 300, 100, 32
NCORE = 8
BLOC = B // NCORE            # batches per core
DPAD = 384                   # D padded to 3*128
NCH = 3
PADROW = V                   # zero row appended to tables
BIAS = 32768                 # int16 index bias
GW = 512                     # tokens per gather group (1 batch)
NG = BLOC                    # gather groups per side
LT = [128, 128, 128, 116]
LT_OFF = [0, 128, 256, 384]
GB = [128, 128, 128, 116]    # gate l-blocks (gate cols 0..499)
LOOP = int(os.environ.get("DAML_LOOP", "1"))


def build_program():
    nc = bacc.Bacc("TRN2", target_bir_lowering=False, debug=False,
                   num_devices=NCORE, num_swdge_queues=4)
    t = {}

    def din(name, shape, dt):
        t[name] = nc.dram_tensor(name, shape, dt, kind="ExternalInput")

    for s in "ui":
        din(f"tab_{s}", (V + 1, DPAD), DT.bfloat16)
        din(f"idx_{s}", (128, 32 * NG), DT.int16)
        din(f"wd_{s}", (128, NCH, 3, 102), DT.bfloat16)
        din(f"dcb_{s}", (102, 1), DT.float32)
        din(f"wabs_{s}", (101, 3, F), DT.bfloat16)
        din(f"wfc_{s}", (F, ID), DT.bfloat16)
        din(f"bfc_{s}", (ID, 1), DT.float32)
        din(f"idrows_{s}", (BLOC, ID), DT.float32)
    din("w3", (128, NCH, 3), DT.bfloat16)
    din("wcb", (1, 1), DT.float32)
    din("ck", (128, 4, 3), DT.bfloat16)
    din("onesd", (1, 512), DT.bfloat16)
    t["out_use"] = nc.dram_tensor("out_use", (BLOC, 2 * ID), DT.float32,
                                  kind="ExternalOutput")
    t["out_item"] = nc.dram_tensor("out_item", (BLOC, 2 * ID), DT.float32,
                                   kind="ExternalOutput")

    with tile.TileContext(nc) as tc:
        _emit(nc, tc, t)

    nc.compile()
    return nc


def _emit(nc, tc, t):
    from contextlib import ExitStack
    from concourse.masks import make_identity
    ctx = ExitStack()

    consts = ctx.enter_context(tc.tile_pool(name="consts", bufs=1))
    et_pool = ctx.enter_context(tc.tile_pool(name="et", bufs=1))
    feat_pool = ctx.enter_context(tc.tile_pool(name="feat", bufs=1))
    gsig_pool = ctx.enter_context(tc.tile_pool(name="gsig", bufs=2))
    lnt_pool = ctx.enter_context(tc.tile_pool(name="lnt", bufs=16))
    att_pool = ctx.enter_context(tc.tile_pool(name="att", bufs=1))
    sm_pool = ctx.enter_context(tc.tile_pool(name="sm", bufs=4))
    gbc_pool = ctx.enter_context(tc.tile_pool(name="gbc", bufs=4))
    u2_pool = ctx.enter_context(tc.tile_pool(name="u2", bufs=2))
    ft_pool = ctx.enter_context(tc.tile_pool(name="ft", bufs=1))

    ps_sq = ctx.enter_context(tc.tile_pool(name="ps_sq", bufs=1, space="PSUM"))
    ps_fps = ctx.enter_context(tc.tile_pool(name="ps_fps", bufs=2, space="PSUM"))
    ps_g = ctx.enter_context(tc.tile_pool(name="ps_g", bufs=1, space="PSUM"))
    ps_small = ctx.enter_context(tc.tile_pool(name="ps_small", bufs=1, space="PSUM"))

    # ---------------- constants / weights ----------------
    idx_sb, wd_sb, wabs_sb, wfc_sb, bfc_sb, dcb_sb, idid_sb = ({} for _ in range(7))
    for s in "ui":
        idx_sb[s] = consts.tile([128, 32 * NG], DT.int16, tag=f"idx{s}", name=f"idx{s}")
        nc.sync.dma_start(out=idx_sb[s][:], in_=t[f"idx_{s}"].ap())
    w3_sb = consts.tile([128, NCH, 3], DT.bfloat16, tag="w3", name="w3")
    nc.sync.dma_start(out=w3_sb[:], in_=t["w3"].ap())
    wcb_sb = consts.tile([1, 1], DT.float32, tag="wcb", name="wcb")
    nc.sync.dma_start(out=wcb_sb[:], in_=t["wcb"].ap())
    wcb_bc = consts.tile([128, 1], DT.float32, tag="wcbb", name="wcbb")
    nc.gpsimd.partition_broadcast(wcb_bc[:, :], wcb_sb[:, :])
    for s in "ui":
        wd_sb[s] = consts.tile([128, NCH, 3, 102], DT.bfloat16, tag=f"wd{s}", name=f"wd{s}")
        nc.sync.dma_start(out=wd_sb[s][:], in_=t[f"wd_{s}"].ap())
        dcb_sb[s] = consts.tile([102, 1], DT.float32, tag=f"dcb{s}", name=f"dcb{s}")
        nc.sync.dma_start(out=dcb_sb[s][:], in_=t[f"dcb_{s}"].ap())
        wabs_sb[s] = consts.tile([101, 3, F], DT.bfloat16, tag=f"wabs{s}", name=f"wabs{s}")
        nc.sync.dma_start(out=wabs_sb[s][:], in_=t[f"wabs_{s}"].ap())
        wfc_sb[s] = consts.tile([F, ID], DT.bfloat16, tag=f"wfc{s}", name=f"wfc{s}")
        nc.sync.dma_start(out=wfc_sb[s][:], in_=t[f"wfc_{s}"].ap())
        bfc_sb[s] = consts.tile([ID, 1], DT.float32, tag=f"bfc{s}", name=f"bfc{s}")
        nc.sync.dma_start(out=bfc_sb[s][:], in_=t[f"bfc_{s}"].ap())

    ck_sb = consts.tile([128, 4, 3], DT.bfloat16, tag="ck", name="ck")
    nc.sync.dma_start(out=ck_sb[:], in_=t["ck"].ap())

    ones_bf = consts.tile([128, 1], DT.bfloat16, tag="ones", name="ones")
    nc.vector.memset(ones_bf[:], 1.0)
    ident_bf = consts.tile([128, 128], DT.bfloat16, tag="identb", name="identb")
    make_identity(nc, ident_bf[:])
    ident_f32 = consts.tile([ID, ID], DT.float32, tag="identf", name="identf")
    make_identity(nc, ident_f32[:])

    # norm-row helper consts (all memset-built):
    # u psum row100 <- -0.5*ones (drain x-2 -> 1), row101 <- -0.125*sum u2
    # i psum row100 <- sum u2 (v2s), row101 <- ones
    onesrow = consts.tile([1, 512], DT.bfloat16, tag="onesrow", name="onesrow")
    nc.vector.memset(onesrow[:, :], 1.0)
    # norm matmuls target psum rows 64..101 (base-64 aligned); cols 36/37
    # of the M=38 lhsT map to psum rows 100/101.
    w6u = consts.tile([100, 38], DT.bfloat16, tag="w6u", name="w6u")
    nc.vector.memset(w6u[:, :], 0.0)
    nc.vector.memset(w6u[:, 37:38], -0.125)
    o6u = consts.tile([1, 38], DT.bfloat16, tag="o6u", name="o6u")
    nc.vector.memset(o6u[:, :], 0.0)
    nc.vector.memset(o6u[:, 36:37], -0.5)
    w6i = consts.tile([100, 38], DT.bfloat16, tag="w6i", name="w6i")
    nc.vector.memset(w6i[:, :], 0.0)
    nc.vector.memset(w6i[:, 36:37], 1.0)
    o6i = consts.tile([1, 38], DT.bfloat16, tag="o6i", name="o6i")
    nc.vector.memset(o6i[:, :], 0.0)
    nc.vector.memset(o6i[:, 37:38], 1.0)

    onesP = consts.tile([128, 500], DT.bfloat16, tag="onesP", name="onesP")
    nc.vector.memset(onesP[:, :], 1.0)

    S_sb = consts.tile([101, 6 * BLOC], DT.bfloat16, tag="Ssb", name="Ssb")
    nc.sync.dma_start(out=S_sb[F:F + 1, :], in_=t["onesd"].ap()[:, 0:6 * BLOC])

    uattT = consts.tile([128, 4, BLOC], DT.float32, tag="uatt", name="uatt")
    iattT = {}

    # feat tiles: rows 0/1 aug, 2..101 feat; cols 500:512 zero (M=128 sq)
    feat_sb = {}
    for s in "ui":
        for b in range(BLOC):
            fsb = feat_pool.tile([102, 512], DT.bfloat16, tag=f"feat{s}{b}",
                                 name=f"feat{s}{b}")
            feat_sb[(s, b)] = fsb
            nc.vector.memset(fsb[:, 500:512], 0.0)

    if LOOP > 1:
        ctx.enter_context(tc.For_i(0, LOOP, 1))

    # ---------------- gathers (one per batch-side) ----------------
    eT = {}
    _gidx = [0]

    def gathers_for(q):
        for g in (2 * q, 2 * q + 1):
            for si, s in enumerate("ui"):
                eT[(s, g)] = et_pool.tile([128, NCH, GW], DT.bfloat16,
                                          tag=f"eT{s}{g}", name=f"eT{s}{g}")
                nc.gpsimd.dma_gather(
                    out_ap=eT[(s, g)][:],
                    in_ap=t[f"tab_{s}"].ap()[BIAS:, :],
                    idxs_ap=idx_sb[s][:, 32 * g:32 * (g + 1)],
                    num_idxs=GW, num_idxs_reg=GW,
                    elem_size=DPAD, transpose=True,
                    queue_num=_gidx[0] % 4,
                )
                _gidx[0] += 1

    # ---------------- building blocks ----------------
    gate_state = {}

    def gate_quad(q):
        # gates for (u,2q),(i,2q),(u,2q+1),(i,2q+1).  Column form: 36 tiny
        # N=1 matmuls accumulate Sum_{tap,chunk} w3*e[l+tap] into oT[l, B]
        # (tap shift applied on the lhsT slice, so no cross-partition
        # combine is needed); sigmoid in column form; 4 aligned PE
        # transposes -> gate row; broadcast; gate eT in place.
        quad = [("u", 2 * q), ("i", 2 * q), ("u", 2 * q + 1), ("i", 2 * q + 1)]
        oT = ps_small.tile([128, 16], DT.float32, tag="rot", name="oT")
        for qi, (s, g) in enumerate(quad):
            for Bb in range(4):
                m = GB[Bb]
                k = 0
                for tp in range(3):
                    for c in range(NCH):
                        nc.tensor.matmul(
                            out=oT[0:m, 4 * qi + Bb:4 * qi + Bb + 1],
                            lhsT=eT[(s, g)][:, c, 128 * Bb + tp:128 * Bb + tp + m],
                            rhs=w3_sb[:, c, tp:tp + 1],
                            start=(k == 0), stop=(k == 8))
                        k += 1
        gate_state[q] = (quad, oT)
        quad, oT = gate_state.pop(q)
        gcol = sm_pool.tile([128, 16], DT.bfloat16, tag="gcol", name="gcol")
        nc.scalar.activation(gcol[:, :], oT[:, :], AF.Sigmoid,
                             bias=wcb_bc[:, :])
        g_row = ps_g.tile([65, 1024], DT.bfloat16, tag="grow", name="grow")
        for qi, (s, g) in enumerate(quad):
            for Bb in range(4):
                m = GB[Bb]
                p0, c0 = (32 * qi, 0) if qi < 3 else (0, 512)
                nc.tensor.matmul(out=g_row[p0:p0 + 1, c0 + 128 * Bb:c0 + 128 * Bb + m],
                                 lhsT=gcol[0:m, 4 * qi + Bb:4 * qi + Bb + 1],
                                 rhs=ident_bf[0:m, 0:m],
                                 is_transpose=True, start=True, stop=True)
        g4s = gsig_pool.tile([65, 1024], DT.bfloat16, tag="gs", name="gs")
        nc.scalar.activation(g4s[:, :], g_row[:, :], AF.Copy)
        for qi, (s, g) in enumerate(quad):
            gb = gbc_pool.tile([128, 500], DT.bfloat16, tag=f"gb{qi}", name="gb")
            p0, c0 = (32 * qi, 0) if qi < 3 else (0, 512)
            nc.gpsimd.partition_broadcast(gb[:, :], g4s[p0:p0 + 1, c0:c0 + 500])
            nc.vector.tensor_tensor(out=eT[(s, g)][:, :, 1:501],
                                    in0=eT[(s, g)][:, :, 1:501],
                                    in1=gb[:, None, :].to_broadcast([128, NCH, 500]),
                                    op=ALU.mult)

    def conv_block(s, b):
        # feat rows 0..99; aug rows 100/101 built in-psum by two extra
        # accumulating matmuls (zero lhsT cols keep feat rows intact):
        #   u: row100 = ones (post x-2), row101 = ||u_l||^2
        #   i: row100 = ||v_m||^2,       row101 = ones
        fps = ps_fps.tile([102, 512], DT.float32, tag="fps", name="fps")
        k = 0
        for tp in range(3):
            for c in range(NCH):
                nc.tensor.matmul(out=fps[0:102, 0:500],
                                 lhsT=wd_sb[s][:, c, tp, :],
                                 rhs=eT[(s, b)][:, c, tp:tp + 500],
                                 start=(k == 0), stop=False,
                                 skip_group_check=True)
                k += 1
        fsb = feat_sb[(s, b)]
        scl = -2.0 if s == "u" else 1.0
        if b < 2:
            nc.scalar.activation(fsb[0:100, 0:500], fps[0:100, 0:500],
                                 AF.Identity, scale=scl,
                                 bias=dcb_sb[s][0:100, :])
        else:
            nc.vector.tensor_scalar(out=fsb[0:100, 0:500], in0=fps[0:100, 0:500],
                                    scalar1=scl, scalar2=dcb_sb[s][0:100, :],
                                    op0=ALU.mult, op1=ALU.add)
        u2t = u2_pool.tile([100, 500], DT.bfloat16, tag="u2", name="u2")
        u2eng = nc.vector if b < 2 else nc.gpsimd
        u2eng.tensor_tensor(out=u2t[0:100, :], in0=fsb[0:100, 0:500],
                            in1=fsb[0:100, 0:500], op=ALU.mult)
        w6, o6 = (w6u, o6u) if s == "u" else (w6i, o6i)
        nc.tensor.matmul(out=fps[64:102, 0:500], lhsT=w6[0:100, :],
                         rhs=u2t[0:100, :], start=False, stop=False,
                         skip_group_check=True)
        nc.tensor.matmul(out=fps[64:102, 0:500], lhsT=o6[0:1, :],
                         rhs=onesrow[0:1, 0:500], start=False, stop=True,
                         skip_group_check=True)
        if b < 2:
            nc.scalar.activation(fsb[64:102, 0:500], fps[64:102, 0:500],
                                 AF.Identity, scale=scl,
                                 bias=dcb_sb[s][64:102, :])
        else:
            nc.vector.tensor_scalar(out=fsb[64:102, 0:500], in0=fps[64:102, 0:500],
                                    scalar1=scl, scalar2=dcb_sb[s][64:102, :],
                                    op0=ALU.mult, op1=ALU.add)

    lnt_live = {}

    def sq_mm(b):
        for half in range(2):
            sq2 = ps_sq.tile([128, 2, 512], DT.float32, tag=f"sq{half}", name="sq")
            for j in range(2):
                lt = 2 * half + j
                nc.tensor.matmul(
                    out=sq2[:, j, 0:500],
                    lhsT=feat_sb[("u", b)][0:102, LT_OFF[lt]:LT_OFF[lt] + 128],
                    rhs=feat_sb[("i", b)][0:102, 0:500],
                    start=True, stop=True)
            lnt_live[(b, half)] = sq2

    def ln_block(b):
        # att = sigmoid(-0.5*ln(sq)) = 1/(1+sqrt(sq)); sq carries both norm
        # terms via the aug rows so Ln needs no bias and merges 2 tiles/op.
        for half in range(2):
            sq2 = lnt_live.pop((b, half))
            lnt = lnt_pool.tile([128, 2, 500], DT.bfloat16, tag="lnt", name="lnt")
            nc.scalar.activation(lnt[:, :, :], sq2[:, :, 0:500], AF.Ln)
            lnt_live[(b, half, "ln")] = lnt

    att_live = {}

    def sig_block(b):
        att = att_pool.tile([128, 4, 500], DT.bfloat16, tag=f"att{b}",
                            name=f"att{b}")
        for half in range(2):
            lnt = lnt_live.pop((b, half, "ln"))
            for j in range(2):
                lt = 2 * half + j
                nc.scalar.activation(att[:, lt, :], lnt[:, j, :],
                                     AF.Sigmoid, scale=-0.5,
                                     accum_out=uattT[:, lt, b:b + 1])
        att_live[b] = att

    def iatt_block(b):
        att = att_live.pop(b)
        ia = ps_small.tile([128, 4], DT.float32, tag="rot", name="ia")
        for ms in range(4):
            for lt in range(4):
                nc.tensor.matmul(
                    out=ia[0:LT[ms], ms:ms + 1],
                    lhsT=att[0:LT[lt], lt, LT_OFF[ms]:LT_OFF[ms] + LT[ms]],
                    rhs=ones_bf[0:LT[lt], 0:1],
                    start=(lt == 0), stop=(lt == 3))
        iab = consts.tile([128, 4], DT.float32, tag=f"iatt{b}", name=f"iatt{b}")
        nc.vector.tensor_copy(iab[:, :], ia[:, :])
        iattT[b] = iab

    fts_live = {}

    def ftp_block(s, b):
        ftp = ps_small.tile([128, 4 * F], DT.bfloat16, tag="rot", name="ftp")
        for lt in range(4):
            m = LT[lt]
            nc.tensor.transpose(
                ftp[:m, F * lt:F * (lt + 1)],
                feat_sb[(s, b)][0:F, LT_OFF[lt]:LT_OFF[lt] + m],
                ident_bf[0:F, 0:F])
        fts = ft_pool.tile([128, 4 * F], DT.bfloat16, tag=f"fts{s}{b % 4}",
                           name="fts")
        nc.vector.tensor_copy(fts[:, :], ftp[:, :])
        fts_live[(s, b)] = fts

    S_cell = []

    def S_alloc():
        S = ps_g.tile([F, 6 * BLOC], DT.float32, tag="grow", name="S")
        S_cell.append(S)
        return S

    def pool_block(s, b):
        si = 0 if s == "u" else 1
        col = 3 * (BLOC * si + b)
        wa = sm_pool.tile([128, 4, 3], DT.bfloat16, tag="wa", name="wa")
        attsrc = uattT[:, :, b:b + 1] if s == "u" else iattT[b][:, :, None]
        nc.vector.tensor_tensor(out=wa[:, :, :],
                                in0=attsrc.to_broadcast([128, 4, 3]),
                                in1=ck_sb[:, :, :], op=ALU.mult)
        fts = fts_live.pop((s, b))
        S_ps = S_cell[0]
        for lt in range(4):
            m = LT[lt]
            nc.tensor.matmul(out=S_ps[:, col:col + 3],
                             lhsT=fts[:m, F * lt:F * (lt + 1)],
                             rhs=wa[:m, lt, :], start=(lt == 0), stop=(lt == 3))

    # ---------------- wave-scheduled driver ----------------
    # stages pin per-engine order; ACT sees contiguous same-table blocks.
    with tc.tile_wait_until(1):
        gathers_for(0)
        gate_quad(0)
        gathers_for(1)
        gate_quad(1)
        gathers_for(2)
        gate_quad(2)
        gathers_for(3)
        gate_quad(3)
        for b in range(BLOC):
            conv_block("u", b)
            conv_block("i", b)
            sq_mm(b)
            if b < 4:
                ln_block(b)
    with tc.tile_wait_until(2):
        for b in range(4):
            sig_block(b)
    with tc.tile_wait_until(3):
        for b in range(4, BLOC):
            ln_block(b)
    with tc.tile_wait_until(4):
        for b in range(4, BLOC):
            sig_block(b)
    with tc.tile_wait_until(1):
        S_ps = S_alloc()
        for b in range(BLOC):
            iatt_block(b)
            ftp_block("u", b), ftp_block("i", b)
            pool_block("u", b), pool_block("i", b)
    with tc.tile_wait_until(4):
        nc.vector.tensor_copy(S_sb[0:F, :], S_cell[0][:, :])

        am_ps = ps_small.tile([F, 2 * BLOC], DT.float32, tag="rot", name="am")
        for si, s in enumerate("ui"):
            for b in range(BLOC):
                for k in range(3):
                    nc.tensor.matmul(
                        out=am_ps[:, BLOC * si + b:BLOC * si + b + 1],
                        lhsT=wabs_sb[s][:, k, :],
                        rhs=S_sb[:, 3 * (BLOC * si + b) + k:3 * (BLOC * si + b) + k + 1],
                        start=(k == 0), stop=(k == 2))
        am_sb = sm_pool.tile([F, 2 * BLOC], DT.bfloat16, tag="am_sb", name="am_sb")
        nc.vector.tensor_copy(am_sb[:, :], am_ps[:, :])

        for si, (s, oname) in enumerate((("u", "out_use"), ("i", "out_item"))):
            fc_ps = ps_small.tile([ID, BLOC], DT.float32, tag="rot", name="fc")
            nc.tensor.matmul(out=fc_ps[:, :], lhsT=wfc_sb[s][:, :],
                             rhs=am_sb[:, BLOC * si:BLOC * (si + 1)],
                             start=True, stop=True)
            fcr = sm_pool.tile([ID, BLOC], DT.float32, tag="fcr", name="fcr")
            nc.scalar.activation(fcr[:, :], fc_ps[:, :], AF.Relu,
                                 bias=bfc_sb[s][:, :])
            fct = ps_small.tile([BLOC, ID], DT.float32, tag="rot", name="fct")
            nc.tensor.transpose(fct[:, :], fcr[:, :], ident_f32[:ID, :ID])
            osb = sm_pool.tile([BLOC, 2 * ID], DT.float32, tag=f"osb{s}", name=f"osb{s}")
            nc.vector.tensor_copy(osb[:, 0:ID], fct[:, :])
            nc.sync.dma_start(out=osb[:, ID:2 * ID], in_=t[f"idrows_{s}"].ap())
            nc.sync.dma_start(out=t[oname].ap(), in_=osb[:, :])

    ctx.close()


# ======================= host side =======================

_PROG = None


def _get_prog():
    global _PROG
    if _PROG is None:
        _PROG = build_program()
    return _PROG


def _bf16_table(tab):
    out = np.zeros((V + 1, DPAD), dtype=BF16)
    out[:V, :D] = np.asarray(tab, dtype=np.float32)
    return out


def _gather_idx(doc):
    """doc: (BLOC, L) ids -> (128, 32*NG) int16 biased index tile"""
    stream = np.full((NG, GW), PADROW, dtype=np.int64)
    for b in range(BLOC):
        stream[b, 1:1 + L] = doc[b]
    biased = (stream - BIAS).astype(np.int16)
    arr = np.zeros((128, 32 * NG), dtype=np.int16)
    for g in range(NG):
        blk = biased[g].reshape(32, 16).T  # idx i -> [i%16, i//16]
        for r in range(8):
            arr[16 * r:16 * (r + 1), 32 * g:32 * (g + 1)] = blk
    return arr


def _window_counts():
    c = np.zeros((3, L), dtype=np.float64)
    for k in range(3):
        for lp in range(k, k + L - 2):
            for d2 in (-1, 0, 1):
                ll = lp + d2
                if 0 <= ll < L:
                    c[k, ll] += 1
    return c


def _prep_weights(inp):
    w = {}
    w3 = np.zeros((DPAD, 3), dtype=np.float32)
    w3[:D, :] = np.asarray(inp["word_cnn_w"][0, 0]).astype(np.float32).T
    w["w3"] = np.ascontiguousarray(w3.reshape(NCH, 128, 3).transpose(1, 0, 2)).astype(BF16)
    w["wcb"] = np.asarray(inp["word_cnn_b"]).astype(np.float32).reshape(1, 1)

    for s, key in (("u", "user"), ("i", "item")):
        dw = np.asarray(inp[f"{key}_doc_cnn_w"]).astype(np.float32)  # (F,1,3,D)
        arr = np.zeros((128, NCH, 3, 102), dtype=BF16)
        for tp in range(3):
            pad = np.zeros((DPAD, F), dtype=np.float32)
            pad[:D] = dw[:, 0, tp, :].T
            arr[:, :, tp, 0:F] = pad.reshape(NCH, 128, F).transpose(1, 0, 2)
        w[f"wd_{s}"] = arr
        dcb = np.asarray(inp[f"{key}_doc_cnn_b"]).astype(np.float32)
        dcb_pad = np.zeros((102, 1), dtype=np.float32)
        dcb_pad[0:100, 0] = dcb * (-2.0 if s == "u" else 1.0)
        w[f"dcb_{s}"] = dcb_pad

        aw = np.asarray(inp[f"{key}_abs_cnn_w"]).astype(np.float32)  # (F,1,3,F)
        ab = np.asarray(inp[f"{key}_abs_cnn_b"]).astype(np.float32)
        scale = (1.0 / (L - 2)) * (-0.5 if s == "u" else 1.0)
        warr = np.zeros((101, 3, F), dtype=BF16)
        for k in range(3):
            warr[:F, k, :] = (aw[:, 0, k, :] * scale).T
        warr[F, 0, :] = ab
        w[f"wabs_{s}"] = warr

        w[f"wfc_{s}"] = np.asarray(inp[f"{key}_fc_w"]).astype(np.float32).T.astype(BF16)
        w[f"bfc_{s}"] = np.asarray(inp[f"{key}_fc_b"]).astype(np.float32).reshape(ID, 1)

    cw = _window_counts()
    ckt = np.zeros((128, 4, 3), dtype=BF16)
    for lt in range(4):
        m = LT[lt]
        ckt[:m, lt, :] = cw[:, LT_OFF[lt]:LT_OFF[lt] + m].T
    w["ck"] = ckt
    return w


def prepare_in_maps(inputs):
    w = _prep_weights(inputs)
    tab_u = _bf16_table(inputs["user_word_emb"])
    tab_i = _bf16_table(inputs["item_word_emb"])
    user_doc = np.asarray(inputs["user_doc"]).astype(np.int64)
    item_doc = np.asarray(inputs["item_doc"]).astype(np.int64)
    uids = np.asarray(inputs["uids"]).astype(np.int64)
    iids = np.asarray(inputs["iids"]).astype(np.int64)
    uid_emb = np.asarray(inputs["uid_emb"]).astype(np.float32)
    iid_emb = np.asarray(inputs["iid_emb"]).astype(np.float32)

    in_maps = []
    for c in range(NCORE):
        sl = slice(BLOC * c, BLOC * (c + 1))
        in_maps.append({
            "tab_u": tab_u, "tab_i": tab_i,
            "idx_u": _gather_idx(user_doc[sl]),
            "idx_i": _gather_idx(item_doc[sl]),
            "w3": w["w3"], "wcb": w["wcb"], "ck": w["ck"],
            "onesd": np.ones((1, 512), dtype=BF16),
            "wd_u": w["wd_u"], "wd_i": w["wd_i"],
            "dcb_u": w["dcb_u"], "dcb_i": w["dcb_i"],
            "wabs_u": w["wabs_u"], "wabs_i": w["wabs_i"],
            "wfc_u": w["wfc_u"], "wfc_i": w["wfc_i"],
            "bfc_u": w["bfc_u"], "bfc_i": w["bfc_i"],
            # crossed on purpose: use_fea carries iid_emb, item_fea uid_emb
            "idrows_u": iid_emb[iids[sl]].astype(np.float32),
            "idrows_i": uid_emb[uids[sl]].astype(np.float32),
        })
    return in_maps


def assemble_outputs(res):
    use = np.concatenate([np.asarray(res.results[c]["out_use"]) for c in range(NCORE)])
    item = np.concatenate([np.asarray(res.results[c]["out_item"]) for c in range(NCORE)])
    return (use.reshape(B, 2, ID).astype(np.float32),
            item.reshape(B, 2, ID).astype(np.float32))


def kernel(**inputs):
    nc = _get_prog()
    in_maps = prepare_in_maps(inputs)
    res = bass_utils.run_bass_kernel_spmd(nc, in_maps, core_ids=list(range(NCORE)))
    return assemble_outputs(res)


# revision 31
# speedup vs baseline: 1.4661x; 1.0105x over previous
"""DAML dense_cnn Trainium2 Bass kernel (v2).

Data-parallel over batch: B=64 -> 8 NeuronCores x 8 batches each.

Per-core pipeline (per side u/i), restructured from v1 for engine balance:
  1. 16 dma_gathers (transpose, bf16, 512 idx each) pull e^T = emb[doc]^T
     into SBUF as (128 dpart, 3 chunks, 512 tok-cols) per batch-side,
     spread over 4 SWDGE queues. int16 index range beaten by biasing the
     table base by 32768 rows (ucode sign-extends). Pad positions gather a
     host-appended zero row.
  2. Gates in column form: per (side,batch) 36 tiny N=1 matmuls (tap
     shift on the lhsT eT slice) accumulate gate[l] on partitions, one
     column per 128-token block; Sigmoid in column form; 4 aligned PE
     transposes -> gate row; gpsimd partition_broadcast; DVE gating mult
     on eT.  PE stream time ~0 vs v1's 144 N=500 matmuls (~20us saved).
  3. conv: 9 matmuls (3 taps x 3 chunks) accumulate feat psum rows 2..101;
     aug rows computed in-psum (row0/1 = ones | sum_f feat^2 via
     K=100 ones-matmul writing psum row directly - no DRAM bounce).
     feat -> sbuf (102, 512) bf16, cols 500:512 zeroed so all sq matmuls
     run M=128.
  4. sq einsum K=102 now includes BOTH norm terms (aug rows), so Ln needs
     no bias -> Ln merged to (128,2,500) ops.  att = sigmoid(-0.5*ln(sq)),
     accum_out gives user row-sums free; item col-sums via N=1
     ones-matmuls (free on PE).
  5. ACT table thrash killed by wave scheduling (tc.tile_wait_until):
     [sig gates x4][Ln b0-3][sig b0-3][Ln b4-7][sig b4-7][relu] = 5 loads.
  6. Pooling: PE transposes of feat + S_k matmuls, abs-conv contraction
     (bias via aug row), fc matmul, ACT relu, PE transpose, id-emb
     indirect gather, DMA out.
"""
import os
import numpy as np
import ml_dtypes

import concourse.bass as bass
import concourse.bacc as bacc
import concourse.tile as tile
from concourse import mybir
from concourse import bass_utils

BF16 = ml_dtypes.bfloat16
DT = mybir.dt
AF = mybir.ActivationFunctionType
ALU = mybir.AluOpType

B, L, V, D, F, ID = 64, 500, 50000, 300, 100, 32
NCORE = 8
BLOC = B // NCORE            # batches per core
DPAD = 384                   # D padded to 3*128
NCH = 3
PADROW = V                   # zero row appended to tables
BIAS = 32768                 # int16 index bias
GW = 512                     # tokens per gather group (1 batch)
NG = BLOC                    # gather groups per side
LT = [128, 128, 128, 116]
LT_OFF = [0, 128, 256, 384]
GB = [128, 128, 128, 116]    # gate l-blocks (gate cols 0..499)
LOOP = int(os.environ.get("DAML_LOOP", "1"))


def build_program():
    nc = bacc.Bacc("TRN2", target_bir_lowering=False, debug=False,
                   num_devices=NCORE, num_swdge_queues=4)
    t = {}

    def din(name, shape, dt):
        t[name] = nc.dram_tensor(name, shape, dt, kind="ExternalInput")

    for s in "ui":
        din(f"tab_{s}", (V + 1, DPAD), DT.bfloat16)
        din(f"idx_{s}", (128, 32 * NG), DT.int16)
        din(f"wd_{s}", (128, NCH, 3, 102), DT.bfloat16)
        din(f"dcb_{s}", (102, 1), DT.float32)
        din(f"wabs_{s}", (101, 3, F), DT.bfloat16)
        din(f"wfc_{s}", (F, ID), DT.bfloat16)
        din(f"bfc_{s}", (ID, 1), DT.float32)
        din(f"idrows_{s}", (BLOC, ID), DT.float32)
    din("w3", (128, NCH, 3), DT.bfloat16)
    din("wcb", (1, 1), DT.float32)
    din("ck", (128, 4, 3), DT.bfloat16)
    din("onesd", (1, 512), DT.bfloat16)
    t["out_use"] = nc.dram_tensor("out_use", (BLOC, 2 * ID), DT.float32,
                                  kind="ExternalOutput")
    t["out_item"] = nc.dram_tensor("out_item", (BLOC, 2 * ID), DT.float32,
                                   kind="ExternalOutput")

    with tile.TileContext(nc) as tc:
        _emit(nc, tc, t)

    nc.compile()
    return nc


def _emit(nc, tc, t):
    from contextlib import ExitStack
    from concourse.masks import make_identity
    ctx = ExitStack()

    consts = ctx.enter_context(tc.tile_pool(name="consts", bufs=1))
    et_pool = ctx.enter_context(tc.tile_pool(name="et", bufs=1))
    feat_pool = ctx.enter_context(tc.tile_pool(name="feat", bufs=1))
    gsig_pool = ctx.enter_context(tc.tile_pool(name="gsig", bufs=2))
    lnt_pool = ctx.enter_context(tc.tile_pool(name="lnt", bufs=16))
    att_pool = ctx.enter_context(tc.tile_pool(name="att", bufs=1))
    sm_pool = ctx.enter_context(tc.tile_pool(name="sm", bufs=4))
    gbc_pool = ctx.enter_context(tc.tile_pool(name="gbc", bufs=4))
    u2_pool = ctx.enter_context(tc.tile_pool(name="u2", bufs=2))
    ft_pool = ctx.enter_context(tc.tile_pool(name="ft", bufs=1))

    ps_sq = ctx.enter_context(tc.tile_pool(name="ps_sq", bufs=1, space="PSUM"))
    ps_fps = ctx.enter_context(tc.tile_pool(name="ps_fps", bufs=2, space="PSUM"))
    ps_g = ctx.enter_context(tc.tile_pool(name="ps_g", bufs=1, space="PSUM"))
    ps_small = ctx.enter_context(tc.tile_pool(name="ps_small", bufs=1, space="PSUM"))

    # ---------------- constants / weights ----------------
    idx_sb, wd_sb, wabs_sb, wfc_sb, bfc_sb, dcb_sb, idid_sb = ({} for _ in range(7))
    for s in "ui":
        idx_sb[s] = consts.tile([128, 32 * NG], DT.int16, tag=f"idx{s}", name=f"idx{s}")
        nc.sync.dma_start(out=idx_sb[s][:], in_=t[f"idx_{s}"].ap())
    w3_sb = consts.tile([128, NCH, 3], DT.bfloat16, tag="w3", name="w3")
    nc.sync.dma_start(out=w3_sb[:], in_=t["w3"].ap())
    wcb_sb = consts.tile([1, 1], DT.float32, tag="wcb", name="wcb")
    nc.sync.dma_start(out=wcb_sb[:], in_=t["wcb"].ap())
    wcb_bc = consts.tile([128, 1], DT.float32, tag="wcbb", name="wcbb")
    nc.gpsimd.partition_broadcast(wcb_bc[:, :], wcb_sb[:, :])
    for s in "ui":
        wd_sb[s] = consts.tile([128, NCH, 3, 102], DT.bfloat16, tag=f"wd{s}", name=f"wd{s}")
        nc.sync.dma_start(out=wd_sb[s][:], in_=t[f"wd_{s}"].ap())
        dcb_sb[s] = consts.tile([102, 1], DT.float32, tag=f"dcb{s}", name=f"dcb{s}")
        nc.sync.dma_start(out=dcb_sb[s][:], in_=t[f"dcb_{s}"].ap())
        wabs_sb[s] = consts.tile([101, 3, F], DT.bfloat16, tag=f"wabs{s}", name=f"wabs{s}")
        nc.sync.dma_start(out=wabs_sb[s][:], in_=t[f"wabs_{s}"].ap())
        wfc_sb[s] = consts.tile([F, ID], DT.bfloat16, tag=f"wfc{s}", name=f"wfc{s}")
        nc.sync.dma_start(out=wfc_sb[s][:], in_=t[f"wfc_{s}"].ap())
        bfc_sb[s] = consts.tile([ID, 1], DT.float32, tag=f"bfc{s}", name=f"bfc{s}")
        nc.sync.dma_start(out=bfc_sb[s][:], in_=t[f"bfc_{s}"].ap())

    ck_sb = consts.tile([128, 4, 3], DT.bfloat16, tag="ck", name="ck")
    nc.sync.dma_start(out=ck_sb[:], in_=t["ck"].ap())

    ones_bf = consts.tile([128, 1], DT.bfloat16, tag="ones", name="ones")
    nc.vector.memset(ones_bf[:], 1.0)
    ident_bf = consts.tile([128, 128], DT.bfloat16, tag="identb", name="identb")
    make_identity(nc, ident_bf[:])
    ident_f32 = consts.tile([ID, ID], DT.float32, tag="identf", name="identf")
    make_identity(nc, ident_f32[:])

    # norm-row helper consts (all memset-built):
    # u psum row100 <- -0.5*ones (drain x-2 -> 1), row101 <- -0.125*sum u2
    # i psum row100 <- sum u2 (v2s), row101 <- ones
    onesrow = consts.tile([1, 512], DT.bfloat16, tag="onesrow", name="onesrow")
    nc.vector.memset(onesrow[:, :], 1.0)
    # norm matmuls target psum rows 64..101 (base-64 aligned); cols 36/37
    # of the M=38 lhsT map to psum rows 100/101.
    w6u = consts.tile([100, 38], DT.bfloat16, tag="w6u", name="w6u")
    nc.vector.memset(w6u[:, :], 0.0)
    nc.vector.memset(w6u[:, 37:38], -0.125)
    o6u = consts.tile([1, 38], DT.bfloat16, tag="o6u", name="o6u")
    nc.vector.memset(o6u[:, :], 0.0)
    nc.vector.memset(o6u[:, 36:37], -0.5)
    w6i = consts.tile([100, 38], DT.bfloat16, tag="w6i", name="w6i")
    nc.vector.memset(w6i[:, :], 0.0)
    nc.vector.memset(w6i[:, 36:37], 1.0)
    o6i = consts.tile([1, 38], DT.bfloat16, tag="o6i", name="o6i")
    nc.vector.memset(o6i[:, :], 0.0)
    nc.vector.memset(o6i[:, 37:38], 1.0)

    onesP = consts.tile([128, 500], DT.bfloat16, tag="onesP", name="onesP")
    nc.vector.memset(onesP[:, :], 1.0)

    S_sb = consts.tile([101, 6 * BLOC], DT.bfloat16, tag="Ssb", name="Ssb")
    nc.sync.dma_start(out=S_sb[F:F + 1, :], in_=t["onesd"].ap()[:, 0:6 * BLOC])

    uattT = consts.tile([128, 4, BLOC], DT.float32, tag="uatt", name="uatt")
    iattT = {}

    # feat tiles: rows 0/1 aug, 2..101 feat; cols 500:512 zero (M=128 sq)
    feat_sb = {}
    for s in "ui":
        for b in range(BLOC):
            fsb = feat_pool.tile([102, 512], DT.bfloat16, tag=f"feat{s}{b}",
                                 name=f"feat{s}{b}")
            feat_sb[(s, b)] = fsb
            nc.vector.memset(fsb[:, 500:512], 0.0)
            if s == "u":
                # pad-col lhsT trick: row0 (ones aug row) = 1 at pad cols so
                # the M=128 sq matmuls give sq = v2s > 0 on pad rows
                # (Ln(0) = -inf/NaN otherwise)
                nc.vector.memset(fsb[0:1, 500:512], 1.0)

    if LOOP > 1:
        ctx.enter_context(tc.For_i(0, LOOP, 1))

    # ---------------- gathers (one per batch-side) ----------------
    eT = {}
    _gidx = [0]

    def gathers_for(q):
        for g in (2 * q, 2 * q + 1):
            for si, s in enumerate("ui"):
                eT[(s, g)] = et_pool.tile([128, NCH, GW], DT.bfloat16,
                                          tag=f"eT{s}{g}", name=f"eT{s}{g}")
                nc.gpsimd.dma_gather(
                    out_ap=eT[(s, g)][:],
                    in_ap=t[f"tab_{s}"].ap()[BIAS:, :],
                    idxs_ap=idx_sb[s][:, 32 * g:32 * (g + 1)],
                    num_idxs=GW, num_idxs_reg=GW,
                    elem_size=DPAD, transpose=True,
                    queue_num=_gidx[0] % 4,
                )
                _gidx[0] += 1

    # ---------------- building blocks ----------------
    gate_state = {}

    def gate_quad(q):
        # gates for (u,2q),(i,2q),(u,2q+1),(i,2q+1).  Column form: 36 tiny
        # N=1 matmuls accumulate Sum_{tap,chunk} w3*e[l+tap] into oT[l, B]
        # (tap shift applied on the lhsT slice, so no cross-partition
        # combine is needed); sigmoid in column form; 4 aligned PE
        # transposes -> gate row; broadcast; gate eT in place.
        quad = [("u", 2 * q), ("i", 2 * q), ("u", 2 * q + 1), ("i", 2 * q + 1)]
        oT = ps_small.tile([128, 16], DT.float32, tag="rot", name="oT")
        for qi, (s, g) in enumerate(quad):
            for Bb in range(4):
                m = GB[Bb]
                k = 0
                for tp in range(3):
                    for c in range(NCH):
                        nc.tensor.matmul(
                            out=oT[0:m, 4 * qi + Bb:4 * qi + Bb + 1],
                            lhsT=eT[(s, g)][:, c, 128 * Bb + tp:128 * Bb + tp + m],
                            rhs=w3_sb[:, c, tp:tp + 1],
                            start=(k == 0), stop=(k == 8))
                        k += 1
        gate_state[q] = (quad, oT)
        quad, oT = gate_state.pop(q)
        gcol = sm_pool.tile([128, 16], DT.bfloat16, tag="gcol", name="gcol")
        nc.scalar.activation(gcol[:, :], oT[:, :], AF.Sigmoid,
                             bias=wcb_bc[:, :])
        g_row = ps_g.tile([65, 1024], DT.bfloat16, tag="grow", name="grow")
        for qi, (s, g) in enumerate(quad):
            for Bb in range(4):
                m = GB[Bb]
                p0, c0 = (32 * qi, 0) if qi < 3 else (0, 512)
                nc.tensor.matmul(out=g_row[p0:p0 + 1, c0 + 128 * Bb:c0 + 128 * Bb + m],
                                 lhsT=gcol[0:m, 4 * qi + Bb:4 * qi + Bb + 1],
                                 rhs=ident_bf[0:m, 0:m],
                                 is_transpose=True, start=True, stop=True)
        g4s = gsig_pool.tile([65, 1024], DT.bfloat16, tag="gs", name="gs")
        nc.scalar.activation(g4s[:, :], g_row[:, :], AF.Copy)
        for qi, (s, g) in enumerate(quad):
            gb = gbc_pool.tile([128, 500], DT.bfloat16, tag=f"gb{qi}", name="gb")
            p0, c0 = (32 * qi, 0) if qi < 3 else (0, 512)
            nc.gpsimd.partition_broadcast(gb[:, :], g4s[p0:p0 + 1, c0:c0 + 500])
            nc.vector.tensor_tensor(out=eT[(s, g)][:, :, 1:501],
                                    in0=eT[(s, g)][:, :, 1:501],
                                    in1=gb[:, None, :].to_broadcast([128, NCH, 500]),
                                    op=ALU.mult)

    def conv_block(s, b):
        # feat rows 0..99; aug rows 100/101 built in-psum by two extra
        # accumulating matmuls (zero lhsT cols keep feat rows intact):
        #   u: row100 = ones (post x-2), row101 = ||u_l||^2
        #   i: row100 = ||v_m||^2,       row101 = ones
        fps = ps_fps.tile([102, 512], DT.float32, tag="fps", name="fps")
        k = 0
        for tp in range(3):
            for c in range(NCH):
                nc.tensor.matmul(out=fps[0:102, 0:500],
                                 lhsT=wd_sb[s][:, c, tp, :],
                                 rhs=eT[(s, b)][:, c, tp:tp + 500],
                                 start=(k == 0), stop=False,
                                 skip_group_check=True)
                k += 1
        fsb = feat_sb[(s, b)]
        scl = -2.0 if s == "u" else 1.0
        if b < 2:
            nc.scalar.activation(fsb[0:100, 0:500], fps[0:100, 0:500],
                                 AF.Identity, scale=scl,
                                 bias=dcb_sb[s][0:100, :])
        else:
            nc.vector.tensor_scalar(out=fsb[0:100, 0:500], in0=fps[0:100, 0:500],
                                    scalar1=scl, scalar2=dcb_sb[s][0:100, :],
                                    op0=ALU.mult, op1=ALU.add)
        u2t = u2_pool.tile([100, 500], DT.bfloat16, tag="u2", name="u2")
        u2eng = nc.vector if b < 2 else nc.gpsimd
        u2eng.tensor_tensor(out=u2t[0:100, :], in0=fsb[0:100, 0:500],
                            in1=fsb[0:100, 0:500], op=ALU.mult)
        w6, o6 = (w6u, o6u) if s == "u" else (w6i, o6i)
        nc.tensor.matmul(out=fps[64:102, 0:500], lhsT=w6[0:100, :],
                         rhs=u2t[0:100, :], start=False, stop=False,
                         skip_group_check=True)
        nc.tensor.matmul(out=fps[64:102, 0:500], lhsT=o6[0:1, :],
                         rhs=onesrow[0:1, 0:500], start=False, stop=True,
                         skip_group_check=True)
        if b < 2:
            nc.scalar.activation(fsb[64:102, 0:500], fps[64:102, 0:500],
                                 AF.Identity, scale=scl,
                                 bias=dcb_sb[s][64:102, :])
        else:
            nc.vector.tensor_scalar(out=fsb[64:102, 0:500], in0=fps[64:102, 0:500],
                                    scalar1=scl, scalar2=dcb_sb[s][64:102, :],
                                    op0=ALU.mult, op1=ALU.add)

    lnt_live = {}

    def sq_mm(b):
        for half in range(2):
            sq2 = ps_sq.tile([128, 2, 512], DT.float32, tag=f"sq{half}", name="sq")
            for j in range(2):
                lt = 2 * half + j
                nc.tensor.matmul(
                    out=sq2[:, j, 0:500],
                    lhsT=feat_sb[("u", b)][0:102, LT_OFF[lt]:LT_OFF[lt] + 128],
                    rhs=feat_sb[("i", b)][0:102, 0:500],
                    start=True, stop=True)
            lnt_live[(b, half)] = sq2

    def ln_block(b):
        # att = sigmoid(-0.5*ln(sq)) = 1/(1+sqrt(sq)); sq carries both norm
        # terms via the aug rows so Ln needs no bias and merges 2 tiles/op.
        for half in range(2):
            sq2 = lnt_live.pop((b, half))
            lnt = lnt_pool.tile([128, 2, 500], DT.bfloat16, tag="lnt", name="lnt")
            nc.scalar.activation(lnt[:, :, :], sq2[:, :, 0:500], AF.Ln)
            lnt_live[(b, half, "ln")] = lnt

    att_live = {}

    def sig_block(b):
        att = att_pool.tile([128, 4, 500], DT.bfloat16, tag=f"att{b}",
                            name=f"att{b}")
        for half in range(2):
            lnt = lnt_live.pop((b, half, "ln"))
            for j in range(2):
                lt = 2 * half + j
                nc.scalar.activation(att[:, lt, :], lnt[:, j, :],
                                     AF.Sigmoid, scale=-0.5,
                                     accum_out=uattT[:, lt, b:b + 1])
        att_live[b] = att

    def iatt_block(b):
        att = att_live.pop(b)
        ia = ps_small.tile([128, 4], DT.float32, tag="rot", name="ia")
        for ms in range(4):
            for lt in range(4):
                nc.tensor.matmul(
                    out=ia[0:LT[ms], ms:ms + 1],
                    lhsT=att[0:LT[lt], lt, LT_OFF[ms]:LT_OFF[ms] + LT[ms]],
                    rhs=ones_bf[0:LT[lt], 0:1],
                    start=(lt == 0), stop=(lt == 3))
        iab = consts.tile([128, 4], DT.float32, tag=f"iatt{b}", name=f"iatt{b}")
        nc.vector.tensor_copy(iab[:, :], ia[:, :])
        iattT[b] = iab

    fts_live = {}

    def ftp_block(s, b):
        ftp = ps_small.tile([128, 4 * F], DT.bfloat16, tag="rot", name="ftp")
        for lt in range(4):
            m = LT[lt]
            nc.tensor.transpose(
                ftp[:m, F * lt:F * (lt + 1)],
                feat_sb[(s, b)][0:F, LT_OFF[lt]:LT_OFF[lt] + m],
                ident_bf[0:F, 0:F])
        fts = ft_pool.tile([128, 4 * F], DT.bfloat16, tag=f"fts{s}{b % 4}",
                           name="fts")
        nc.vector.tensor_copy(fts[:, :], ftp[:, :])
        fts_live[(s, b)] = fts

    S_cell = []

    def S_alloc():
        S = ps_g.tile([F, 6 * BLOC], DT.float32, tag="grow", name="S")
        S_cell.append(S)
        return S

    def pool_block(s, b):
        si = 0 if s == "u" else 1
        col = 3 * (BLOC * si + b)
        wa = sm_pool.tile([128, 4, 3], DT.bfloat16, tag="wa", name="wa")
        attsrc = uattT[:, :, b:b + 1] if s == "u" else iattT[b][:, :, None]
        nc.vector.tensor_tensor(out=wa[:, :, :],
                                in0=attsrc.to_broadcast([128, 4, 3]),
                                in1=ck_sb[:, :, :], op=ALU.mult)
        fts = fts_live.pop((s, b))
        S_ps = S_cell[0]
        for lt in range(4):
            m = LT[lt]
            nc.tensor.matmul(out=S_ps[:, col:col + 3],
                             lhsT=fts[:m, F * lt:F * (lt + 1)],
                             rhs=wa[:m, lt, :], start=(lt == 0), stop=(lt == 3))

    # ---------------- wave-scheduled driver ----------------
    # stages pin per-engine order; ACT sees contiguous same-table blocks.
    with tc.tile_wait_until(1):
        gathers_for(0)
        gate_quad(0)
        gathers_for(1)
        gate_quad(1)
        gathers_for(2)
        gate_quad(2)
        gathers_for(3)
        gate_quad(3)
        for b in range(BLOC):
            conv_block("u", b)
            conv_block("i", b)
            sq_mm(b)
            if b < 4:
                ln_block(b)
    with tc.tile_wait_until(2):
        for b in range(4):
            sig_block(b)
    with tc.tile_wait_until(3):
        for b in range(4, BLOC):
            ln_block(b)
    with tc.tile_wait_until(4):
        for b in range(4, BLOC):
            sig_block(b)
    with tc.tile_wait_until(1):
        S_ps = S_alloc()
        for b in range(BLOC):
            iatt_block(b)
            ftp_block("u", b), ftp_block("i", b)
            pool_block("u", b), pool_block("i", b)
    with tc.tile_wait_until(4):
        nc.vector.tensor_copy(S_sb[0:F, :], S_cell[0][:, :])

        am_ps = ps_small.tile([F, 2 * BLOC], DT.float32, tag="rot", name="am")
        for si, s in enumerate("ui"):
            for b in range(BLOC):
                for k in range(3):
                    nc.tensor.matmul(
                        out=am_ps[:, BLOC * si + b:BLOC * si + b + 1],
                        lhsT=wabs_sb[s][:, k, :],
                        rhs=S_sb[:, 3 * (BLOC * si + b) + k:3 * (BLOC * si + b) + k + 1],
                        start=(k == 0), stop=(k == 2))
        am_sb = sm_pool.tile([F, 2 * BLOC], DT.bfloat16, tag="am_sb", name="am_sb")
        nc.vector.tensor_copy(am_sb[:, :], am_ps[:, :])

        for si, (s, oname) in enumerate((("u", "out_use"), ("i", "out_item"))):
            fc_ps = ps_small.tile([ID, BLOC], DT.float32, tag="rot", name="fc")
            nc.tensor.matmul(out=fc_ps[:, :], lhsT=wfc_sb[s][:, :],
                             rhs=am_sb[:, BLOC * si:BLOC * (si + 1)],
                             start=True, stop=True)
            fcr = sm_pool.tile([ID, BLOC], DT.float32, tag="fcr", name="fcr")
            nc.scalar.activation(fcr[:, :], fc_ps[:, :], AF.Relu,
                                 bias=bfc_sb[s][:, :])
            fct = ps_small.tile([BLOC, ID], DT.float32, tag="rot", name="fct")
            nc.tensor.transpose(fct[:, :], fcr[:, :], ident_f32[:ID, :ID])
            osb = sm_pool.tile([BLOC, 2 * ID], DT.float32, tag=f"osb{s}", name=f"osb{s}")
            nc.vector.tensor_copy(osb[:, 0:ID], fct[:, :])
            nc.sync.dma_start(out=osb[:, ID:2 * ID], in_=t[f"idrows_{s}"].ap())
            nc.sync.dma_start(out=t[oname].ap(), in_=osb[:, :])

    ctx.close()


# ======================= host side =======================

_PROG = None


def _get_prog():
    global _PROG
    if _PROG is None:
        _PROG = build_program()
    return _PROG


def _bf16_table(tab):
    out = np.zeros((V + 1, DPAD), dtype=BF16)
    out[:V, :D] = np.asarray(tab, dtype=np.float32)
    return out


def _gather_idx(doc):
    """doc: (BLOC, L) ids -> (128, 32*NG) int16 biased index tile"""
    stream = np.full((NG, GW), PADROW, dtype=np.int64)
    for b in range(BLOC):
        stream[b, 1:1 + L] = doc[b]
    biased = (stream - BIAS).astype(np.int16)
    arr = np.zeros((128, 32 * NG), dtype=np.int16)
    for g in range(NG):
        blk = biased[g].reshape(32, 16).T  # idx i -> [i%16, i//16]
        for r in range(8):
            arr[16 * r:16 * (r + 1), 32 * g:32 * (g + 1)] = blk
    return arr


def _window_counts():
    c = np.zeros((3, L), dtype=np.float64)
    for k in range(3):
        for lp in range(k, k + L - 2):
            for d2 in (-1, 0, 1):
                ll = lp + d2
                if 0 <= ll < L:
                    c[k, ll] += 1
    return c


def _prep_weights(inp):
    w = {}
    w3 = np.zeros((DPAD, 3), dtype=np.float32)
    w3[:D, :] = np.asarray(inp["word_cnn_w"][0, 0]).astype(np.float32).T
    w["w3"] = np.ascontiguousarray(w3.reshape(NCH, 128, 3).transpose(1, 0, 2)).astype(BF16)
    w["wcb"] = np.asarray(inp["word_cnn_b"]).astype(np.float32).reshape(1, 1)

    for s, key in (("u", "user"), ("i", "item")):
        dw = np.asarray(inp[f"{key}_doc_cnn_w"]).astype(np.float32)  # (F,1,3,D)
        arr = np.zeros((128, NCH, 3, 102), dtype=BF16)
        for tp in range(3):
            pad = np.zeros((DPAD, F), dtype=np.float32)
            pad[:D] = dw[:, 0, tp, :].T
            arr[:, :, tp, 0:F] = pad.reshape(NCH, 128, F).transpose(1, 0, 2)
        w[f"wd_{s}"] = arr
        dcb = np.asarray(inp[f"{key}_doc_cnn_b"]).astype(np.float32)
        dcb_pad = np.zeros((102, 1), dtype=np.float32)
        dcb_pad[0:100, 0] = dcb * (-2.0 if s == "u" else 1.0)
        w[f"dcb_{s}"] = dcb_pad

        aw = np.asarray(inp[f"{key}_abs_cnn_w"]).astype(np.float32)  # (F,1,3,F)
        ab = np.asarray(inp[f"{key}_abs_cnn_b"]).astype(np.float32)
        scale = (1.0 / (L - 2)) * (-0.5 if s == "u" else 1.0)
        warr = np.zeros((101, 3, F), dtype=BF16)
        for k in range(3):
            warr[:F, k, :] = (aw[:, 0, k, :] * scale).T
        warr[F, 0, :] = ab
        w[f"wabs_{s}"] = warr

        w[f"wfc_{s}"] = np.asarray(inp[f"{key}_fc_w"]).astype(np.float32).T.astype(BF16)
        w[f"bfc_{s}"] = np.asarray(inp[f"{key}_fc_b"]).astype(np.float32).reshape(ID, 1)

    cw = _window_counts()
    ckt = np.zeros((128, 4, 3), dtype=BF16)
    for lt in range(4):
        m = LT[lt]
        ckt[:m, lt, :] = cw[:, LT_OFF[lt]:LT_OFF[lt] + m].T
    w["ck"] = ckt
    return w


def prepare_in_maps(inputs):
    w = _prep_weights(inputs)
    tab_u = _bf16_table(inputs["user_word_emb"])
    tab_i = _bf16_table(inputs["item_word_emb"])
    user_doc = np.asarray(inputs["user_doc"]).astype(np.int64)
    item_doc = np.asarray(inputs["item_doc"]).astype(np.int64)
    uids = np.asarray(inputs["uids"]).astype(np.int64)
    iids = np.asarray(inputs["iids"]).astype(np.int64)
    uid_emb = np.asarray(inputs["uid_emb"]).astype(np.float32)
    iid_emb = np.asarray(inputs["iid_emb"]).astype(np.float32)

    in_maps = []
    for c in range(NCORE):
        sl = slice(BLOC * c, BLOC * (c + 1))
        in_maps.append({
            "tab_u": tab_u, "tab_i": tab_i,
            "idx_u": _gather_idx(user_doc[sl]),
            "idx_i": _gather_idx(item_doc[sl]),
            "w3": w["w3"], "wcb": w["wcb"], "ck": w["ck"],
            "onesd": np.ones((1, 512), dtype=BF16),
            "wd_u": w["wd_u"], "wd_i": w["wd_i"],
            "dcb_u": w["dcb_u"], "dcb_i": w["dcb_i"],
            "wabs_u": w["wabs_u"], "wabs_i": w["wabs_i"],
            "wfc_u": w["wfc_u"], "wfc_i": w["wfc_i"],
            "bfc_u": w["bfc_u"], "bfc_i": w["bfc_i"],
            # crossed on purpose: use_fea carries iid_emb, item_fea uid_emb
            "idrows_u": iid_emb[iids[sl]].astype(np.float32),
            "idrows_i": uid_emb[uids[sl]].astype(np.float32),
        })
    return in_maps


def assemble_outputs(res):
    use = np.concatenate([np.asarray(res.results[c]["out_use"]) for c in range(NCORE)])
    item = np.concatenate([np.asarray(res.results[c]["out_item"]) for c in range(NCORE)])
    return (use.reshape(B, 2, ID).astype(np.float32),
            item.reshape(B, 2, ID).astype(np.float32))


def kernel(**inputs):
    nc = _get_prog()
    in_maps = prepare_in_maps(inputs)
    res = bass_utils.run_bass_kernel_spmd(nc, in_maps, core_ids=list(range(NCORE)))
    return assemble_outputs(res)


# revision 41
# speedup vs baseline: 1.4709x; 1.0032x over previous
"""DAML dense_cnn Trainium2 Bass kernel (v2).

Data-parallel over batch: B=64 -> 8 NeuronCores x 8 batches each.

Per-core pipeline (per side u/i), restructured from v1 for engine balance:
  1. 16 dma_gathers (transpose, bf16, 512 idx each) pull e^T = emb[doc]^T
     into SBUF as (128 dpart, 3 chunks, 512 tok-cols) per batch-side,
     spread over 4 SWDGE queues. int16 index range beaten by biasing the
     table base by 32768 rows (ucode sign-extends). Pad positions gather a
     host-appended zero row.
  2. Gates in column form: per (side,batch) 36 tiny N=1 matmuls (tap
     shift on the lhsT eT slice) accumulate gate[l] on partitions, one
     column per 128-token block; Sigmoid in column form; 4 aligned PE
     transposes -> gate row; gpsimd partition_broadcast; DVE gating mult
     on eT.  PE stream time ~0 vs v1's 144 N=500 matmuls (~20us saved).
  3. conv: 9 matmuls (3 taps x 3 chunks) accumulate feat psum rows 2..101;
     aug rows computed in-psum (row0/1 = ones | sum_f feat^2 via
     K=100 ones-matmul writing psum row directly - no DRAM bounce).
     feat -> sbuf (102, 512) bf16, cols 500:512 zeroed so all sq matmuls
     run M=128.
  4. sq einsum K=102 now includes BOTH norm terms (aug rows), so Ln needs
     no bias -> Ln merged to (128,2,500) ops.  att = sigmoid(-0.5*ln(sq)),
     accum_out gives user row-sums free; item col-sums via N=1
     ones-matmuls (free on PE).
  5. ACT table thrash killed by wave scheduling (tc.tile_wait_until):
     [sig gates x4][Ln b0-3][sig b0-3][Ln b4-7][sig b4-7][relu] = 5 loads.
  6. Pooling: PE transposes of feat + S_k matmuls, abs-conv contraction
     (bias via aug row), fc matmul, ACT relu, PE transpose, id-emb
     indirect gather, DMA out.
"""
import os
import numpy as np
import ml_dtypes

import concourse.bass as bass
import concourse.bacc as bacc
import concourse.tile as tile
from concourse import mybir
from concourse import bass_utils

BF16 = ml_dtypes.bfloat16
DT = mybir.dt
AF = mybir.ActivationFunctionType
ALU = mybir.AluOpType

B, L, V, D, F, ID = 64, 500, 50000, 300, 100, 32
NCORE = 8
BLOC = B // NCORE            # batches per core
DPAD = 384                   # D padded to 3*128
NCH = 3
PADROW = 32760 if os.environ.get("DAML_NOBIAS") else V   # zero row
BIAS = 0 if os.environ.get("DAML_NOBIAS") else 32768   # int16 index bias
GW = 512                     # tokens per gather group (1 batch)
NG = BLOC                    # gather groups per side
LT = [128, 128, 128, 116]
LT_OFF = [0, 128, 256, 384]
GB = [128, 128, 128, 116]    # gate l-blocks (gate cols 0..499)
SIM_INIT = os.environ.get("DAML_SIM_INIT") == "1"   # CoreSim-only psum memsets
LOOP = int(os.environ.get("DAML_LOOP", "1"))


def build_program():
    nc = bacc.Bacc("TRN2", target_bir_lowering=False, debug=False,
                   num_devices=NCORE, num_swdge_queues=4)
    t = {}

    def din(name, shape, dt):
        t[name] = nc.dram_tensor(name, shape, dt, kind="ExternalInput")

    for s in "ui":
        din(f"tab_{s}", (V + 1, DPAD), DT.bfloat16)
        din(f"idx_{s}", (128, 32 * NG), DT.int16)
        din(f"wd_{s}", (128, NCH, 3, 102), DT.bfloat16)
        din(f"dcb_{s}", (102, 1), DT.float32)
        din(f"wabs_{s}", (101, 3, F), DT.bfloat16)
        din(f"wfc_{s}", (F, ID), DT.bfloat16)
        din(f"bfc_{s}", (ID, 1), DT.float32)
        din(f"idrows_{s}", (BLOC, ID), DT.float32)
    din("w3", (128, NCH, 3), DT.bfloat16)
    din("wcb", (1, 1), DT.float32)
    din("ck", (128, 4, 3), DT.bfloat16)
    din("onesd", (1, 512), DT.bfloat16)
    t["dbg"] = nc.dram_tensor("dbg", (128, 4096), DT.float32,
                              kind="ExternalOutput")
    t["out_use"] = nc.dram_tensor("out_use", (BLOC, 2 * ID), DT.float32,
                                  kind="ExternalOutput")
    t["out_item"] = nc.dram_tensor("out_item", (BLOC, 2 * ID), DT.float32,
                                   kind="ExternalOutput")

    with tile.TileContext(nc) as tc:
        _emit(nc, tc, t)

    nc.compile()
    return nc


def _emit(nc, tc, t):
    from contextlib import ExitStack
    from concourse.masks import make_identity
    ctx = ExitStack()

    consts = ctx.enter_context(tc.tile_pool(name="consts", bufs=1))
    et_pool = ctx.enter_context(tc.tile_pool(name="et", bufs=1))
    feat_pool = ctx.enter_context(tc.tile_pool(name="feat", bufs=1))
    gsig_pool = ctx.enter_context(tc.tile_pool(name="gsig", bufs=2))
    lnt_pool = ctx.enter_context(tc.tile_pool(name="lnt", bufs=16))
    att_pool = ctx.enter_context(tc.tile_pool(name="att", bufs=1))
    sm_pool = ctx.enter_context(tc.tile_pool(name="sm", bufs=4))
    gbc_pool = ctx.enter_context(tc.tile_pool(name="gbc", bufs=4))
    u2_pool = ctx.enter_context(tc.tile_pool(name="u2", bufs=2))
    ft_pool = ctx.enter_context(tc.tile_pool(name="ft", bufs=1))

    ps_sq = ctx.enter_context(tc.tile_pool(name="ps_sq", bufs=1, space="PSUM"))
    ps_fps = ctx.enter_context(tc.tile_pool(name="ps_fps", bufs=2, space="PSUM"))
    ps_g = ctx.enter_context(tc.tile_pool(name="ps_g", bufs=1, space="PSUM"))
    ps_small = ctx.enter_context(tc.tile_pool(name="ps_small", bufs=1, space="PSUM"))

    # ---------------- constants / weights ----------------
    idx_sb, wd_sb, wabs_sb, wfc_sb, bfc_sb, dcb_sb, idid_sb = ({} for _ in range(7))
    for s in "ui":
        idx_sb[s] = consts.tile([128, 32 * NG], DT.int16, tag=f"idx{s}", name=f"idx{s}")
        nc.sync.dma_start(out=idx_sb[s][:], in_=t[f"idx_{s}"].ap())
    w3_sb = consts.tile([128, NCH, 3], DT.bfloat16, tag="w3", name="w3")
    nc.sync.dma_start(out=w3_sb[:], in_=t["w3"].ap())
    wcb_sb = consts.tile([1, 1], DT.float32, tag="wcb", name="wcb")
    nc.sync.dma_start(out=wcb_sb[:], in_=t["wcb"].ap())
    wcb_bc = consts.tile([128, 1], DT.float32, tag="wcbb", name="wcbb")
    nc.gpsimd.partition_broadcast(wcb_bc[:, :], wcb_sb[:, :])
    for s in "ui":
        wd_sb[s] = consts.tile([128, NCH, 3, 102], DT.bfloat16, tag=f"wd{s}", name=f"wd{s}")
        nc.sync.dma_start(out=wd_sb[s][:], in_=t[f"wd_{s}"].ap())
        dcb_sb[s] = consts.tile([102, 1], DT.float32, tag=f"dcb{s}", name=f"dcb{s}")
        nc.sync.dma_start(out=dcb_sb[s][:], in_=t[f"dcb_{s}"].ap())
        wabs_sb[s] = consts.tile([101, 3, F], DT.bfloat16, tag=f"wabs{s}", name=f"wabs{s}")
        nc.sync.dma_start(out=wabs_sb[s][:], in_=t[f"wabs_{s}"].ap())
        wfc_sb[s] = consts.tile([F, ID], DT.bfloat16, tag=f"wfc{s}", name=f"wfc{s}")
        nc.sync.dma_start(out=wfc_sb[s][:], in_=t[f"wfc_{s}"].ap())
        bfc_sb[s] = consts.tile([ID, 1], DT.float32, tag=f"bfc{s}", name=f"bfc{s}")
        nc.sync.dma_start(out=bfc_sb[s][:], in_=t[f"bfc_{s}"].ap())

    ck_sb = consts.tile([128, 4, 3], DT.bfloat16, tag="ck", name="ck")
    nc.sync.dma_start(out=ck_sb[:], in_=t["ck"].ap())

    ones_bf = consts.tile([128, 1], DT.bfloat16, tag="ones", name="ones")
    nc.vector.memset(ones_bf[:], 1.0)
    ident_bf = consts.tile([128, 128], DT.bfloat16, tag="identb", name="identb")
    make_identity(nc, ident_bf[:])
    ident_f32 = consts.tile([ID, ID], DT.float32, tag="identf", name="identf")
    make_identity(nc, ident_f32[:])

    # norm-row helper consts (all memset-built):
    # u psum row100 <- -0.5*ones (drain x-2 -> 1), row101 <- -0.125*sum u2
    # i psum row100 <- sum u2 (v2s), row101 <- ones
    onesrow = consts.tile([1, 512], DT.bfloat16, tag="onesrow", name="onesrow")
    nc.vector.memset(onesrow[:, :], 1.0)
    # norm matmuls target psum rows 64..101 (base-64 aligned); cols 36/37
    # of the M=38 lhsT map to psum rows 100/101.
    w6u = consts.tile([100, 38], DT.bfloat16, tag="w6u", name="w6u")
    nc.vector.memset(w6u[:, :], 0.0)
    nc.vector.memset(w6u[:, 37:38], -0.125)
    o6u = consts.tile([1, 38], DT.bfloat16, tag="o6u", name="o6u")
    nc.vector.memset(o6u[:, :], 0.0)
    nc.vector.memset(o6u[:, 36:37], -0.5)
    w6i = consts.tile([100, 38], DT.bfloat16, tag="w6i", name="w6i")
    nc.vector.memset(w6i[:, :], 0.0)
    nc.vector.memset(w6i[:, 36:37], 1.0)
    o6i = consts.tile([1, 38], DT.bfloat16, tag="o6i", name="o6i")
    nc.vector.memset(o6i[:, :], 0.0)
    nc.vector.memset(o6i[:, 37:38], 1.0)

    onesP = consts.tile([128, 500], DT.bfloat16, tag="onesP", name="onesP")
    nc.vector.memset(onesP[:, :], 1.0)

    S_sb = consts.tile([101, 6 * BLOC], DT.bfloat16, tag="Ssb", name="Ssb")
    nc.sync.dma_start(out=S_sb[F:F + 1, :], in_=t["onesd"].ap()[:, 0:6 * BLOC])

    uattT = consts.tile([128, 4, BLOC], DT.float32, tag="uatt", name="uatt")
    iattT = {}

    # feat tiles: rows 0/1 aug, 2..101 feat; cols 500:512 zero (M=128 sq)
    feat_sb = {}
    for s in "ui":
        for b in range(BLOC):
            fsb = feat_pool.tile([102, 512], DT.bfloat16, tag=f"feat{s}{b}",
                                 name=f"feat{s}{b}")
            feat_sb[(s, b)] = fsb
            nc.vector.memset(fsb[:, 500:512], 0.0)
            if s == "u":
                # pad-col lhsT trick: row0 (ones aug row) = 1 at pad cols so
                # the M=128 sq matmuls give sq = v2s > 0 on pad rows
                # (Ln(0) = -inf/NaN otherwise)
                nc.vector.memset(fsb[0:1, 500:512], 1.0)

    if LOOP > 1:
        ctx.enter_context(tc.For_i(0, LOOP, 1))

    # ---------------- gathers (one per batch-side) ----------------
    eT = {}
    _gidx = [0]

    def gathers_for(q):
        for g in (2 * q, 2 * q + 1):
            for si, s in enumerate("ui"):
                eT[(s, g)] = et_pool.tile([128, NCH, GW], DT.bfloat16,
                                          tag=f"eT{s}{g}", name=f"eT{s}{g}")
                nc.gpsimd.dma_gather(
                    out_ap=eT[(s, g)][:],
                    in_ap=t[f"tab_{s}"].ap()[BIAS:, :],
                    idxs_ap=idx_sb[s][:, 32 * g:32 * (g + 1)],
                    num_idxs=GW, num_idxs_reg=GW,
                    elem_size=DPAD, transpose=True,
                    queue_num=0,
                )
                _gidx[0] += 1

    # ---------------- building blocks ----------------
    gate_state = {}

    def gate_quad(q):
        # gates for (u,2q),(i,2q),(u,2q+1),(i,2q+1).  Column form: 36 tiny
        # N=1 matmuls accumulate Sum_{tap,chunk} w3*e[l+tap] into oT[l, B]
        # (tap shift applied on the lhsT slice, so no cross-partition
        # combine is needed); sigmoid in column form; 4 aligned PE
        # transposes -> gate row; broadcast; gate eT in place.
        quad = [("u", 2 * q), ("i", 2 * q), ("u", 2 * q + 1), ("i", 2 * q + 1)]
        oT = ps_small.tile([128, 16], DT.float32, tag="rot", name="oT")
        if SIM_INIT:
            # rows GB[3]..127 of the Bb=3 columns are never matmul-written;
            # harmless junk on HW (excluded downstream), but CoreSim's
            # uninit tracker needs them defined
            nc.vector.memset(oT[:, :], 0.0)
        for qi, (s, g) in enumerate(quad):
            for Bb in range(4):
                m = GB[Bb]
                k = 0
                for tp in range(3):
                    for c in range(NCH):
                        nc.tensor.matmul(
                            out=oT[0:m, 4 * qi + Bb:4 * qi + Bb + 1],
                            lhsT=eT[(s, g)][:, c, 128 * Bb + tp:128 * Bb + tp + m],
                            rhs=w3_sb[:, c, tp:tp + 1],
                            start=(k == 0), stop=(k == 8))
                        k += 1
        gate_state[q] = (quad, oT)
        quad, oT = gate_state.pop(q)
        gcol = sm_pool.tile([128, 16], DT.bfloat16, tag="gcol", name="gcol")
        nc.scalar.activation(gcol[:, :], oT[:, :], AF.Sigmoid,
                             bias=wcb_bc[:, :])
        g_row = ps_g.tile([65, 1024], DT.bfloat16, tag="grow", name="grow")
        if SIM_INIT:
            nc.vector.memset(g_row[:, :], 0.0)
        for qi, (s, g) in enumerate(quad):
            for Bb in range(4):
                m = GB[Bb]
                p0, c0 = (32 * qi, 0) if qi < 3 else (0, 512)
                nc.tensor.matmul(out=g_row[p0:p0 + 1, c0 + 128 * Bb:c0 + 128 * Bb + m],
                                 lhsT=gcol[0:m, 4 * qi + Bb:4 * qi + Bb + 1],
                                 rhs=ident_bf[0:m, 0:m],
                                 is_transpose=True, start=True, stop=True)
        g4s = gsig_pool.tile([65, 1024], DT.bfloat16, tag="gs", name="gs")
        nc.scalar.activation(g4s[:, :], g_row[:, :], AF.Copy)
        if q == 0:
            dbg_g = consts.tile([65, 1024], DT.float32, tag="dbg_g", name="dbg_g")
            nc.vector.tensor_copy(dbg_g[:, :], g4s[:, :])
            nc.sync.dma_start(out=t["dbg"].ap()[0:65, 2048:3072], in_=dbg_g[:, :])
        for qi, (s, g) in enumerate(quad):
            gb = gbc_pool.tile([128, 500], DT.bfloat16, tag=f"gb{qi}", name="gb")
            p0, c0 = (32 * qi, 0) if qi < 3 else (0, 512)
            nc.gpsimd.partition_broadcast(gb[:, :], g4s[p0:p0 + 1, c0:c0 + 500])
            nc.vector.tensor_tensor(out=eT[(s, g)][:, :, 1:501],
                                    in0=eT[(s, g)][:, :, 1:501],
                                    in1=gb[:, None, :].to_broadcast([128, NCH, 500]),
                                    op=ALU.mult)
            if s == "i" and g == 0:
                dbg_e = consts.tile([128, 512], DT.float32, tag="dbg_e", name="dbg_e")
                nc.vector.tensor_copy(dbg_e[:, :], eT[(s, g)][:, 0, :])
                nc.sync.dma_start(out=t["dbg"].ap()[:, 3072:3584], in_=dbg_e[:, :])

    def conv_block(s, b):
        # feat rows 0..99; aug rows 100/101 built in-psum by two extra
        # accumulating matmuls (zero lhsT cols keep feat rows intact):
        #   u: row100 = ones (post x-2), row101 = ||u_l||^2
        #   i: row100 = ||v_m||^2,       row101 = ones
        fps = ps_fps.tile([102, 512], DT.float32, tag="fps", name="fps")
        k = 0
        for tp in range(3):
            for c in range(NCH):
                nc.tensor.matmul(out=fps[0:102, 0:500],
                                 lhsT=wd_sb[s][:, c, tp, :],
                                 rhs=eT[(s, b)][:, c, tp:tp + 500],
                                 start=(k == 0), stop=False,
                                 skip_group_check=True)
                k += 1
        fsb = feat_sb[(s, b)]
        scl = -2.0 if s == "u" else 1.0
        if False:
            nc.scalar.activation(fsb[0:100, 0:500], fps[0:100, 0:500],
                                 AF.Identity, scale=scl,
                                 bias=dcb_sb[s][0:100, :])
        else:
            nc.vector.tensor_scalar(out=fsb[0:100, 0:500], in0=fps[0:100, 0:500],
                                    scalar1=scl, scalar2=dcb_sb[s][0:100, :],
                                    op0=ALU.mult, op1=ALU.add)
        u2t = u2_pool.tile([100, 500], DT.bfloat16, tag="u2", name="u2")
        u2eng = nc.vector if b < 2 else nc.gpsimd
        u2eng.tensor_tensor(out=u2t[0:100, :], in0=fsb[0:100, 0:500],
                            in1=fsb[0:100, 0:500], op=ALU.mult)
        w6, o6 = (w6u, o6u) if s == "u" else (w6i, o6i)
        nc.tensor.matmul(out=fps[64:102, 0:500], lhsT=w6[0:100, :],
                         rhs=u2t[0:100, :], start=False, stop=False,
                         skip_group_check=True)
        nc.tensor.matmul(out=fps[64:102, 0:500], lhsT=o6[0:1, :],
                         rhs=onesrow[0:1, 0:500], start=False, stop=True,
                         skip_group_check=True)
        if False:
            nc.scalar.activation(fsb[64:102, 0:500], fps[64:102, 0:500],
                                 AF.Identity, scale=scl,
                                 bias=dcb_sb[s][64:102, :])
        else:
            nc.vector.tensor_scalar(out=fsb[64:102, 0:500], in0=fps[64:102, 0:500],
                                    scalar1=scl, scalar2=dcb_sb[s][64:102, :],
                                    op0=ALU.mult, op1=ALU.add)

    lnt_live = {}

    def sq_mm(b):
        for half in range(2):
            sq2 = ps_sq.tile([128, 2, 512], DT.float32, tag=f"sq{half}", name="sq")
            for j in range(2):
                lt = 2 * half + j
                nc.tensor.matmul(
                    out=sq2[:, j, 0:500],
                    lhsT=feat_sb[("u", b)][0:102, LT_OFF[lt]:LT_OFF[lt] + 128],
                    rhs=feat_sb[("i", b)][0:102, 0:500],
                    start=True, stop=True)
            lnt_live[(b, half)] = sq2

    def ln_block(b):
        # att = sigmoid(-0.5*ln(sq)) = 1/(1+sqrt(sq)); sq carries both norm
        # terms via the aug rows so Ln needs no bias and merges 2 tiles/op.
        for half in range(2):
            sq2 = lnt_live.pop((b, half))
            lnt = lnt_pool.tile([128, 2, 500], DT.bfloat16, tag="lnt", name="lnt")
            nc.scalar.activation(lnt[:, :, :], sq2[:, :, 0:500], AF.Ln)
            lnt_live[(b, half, "ln")] = lnt

    att_live = {}
    att_dbg_cell = []

    def sig_block(b):
        att = att_pool.tile([128, 4, 500], DT.bfloat16, tag=f"att{b}",
                            name=f"att{b}")
        for half in range(2):
            lnt = lnt_live.pop((b, half, "ln"))
            for j in range(2):
                lt = 2 * half + j
                nc.scalar.activation(att[:, lt, :], lnt[:, j, :],
                                     AF.Sigmoid, scale=-0.5,
                                     accum_out=uattT[:, lt, b:b + 1])
        att_live[b] = att
        if b == 0:
            att_dbg_cell.append(att)

    def iatt_block(b):
        att = att_live.pop(b)
        ia = ps_small.tile([128, 4], DT.float32, tag="rot", name="ia")
        if SIM_INIT:
            nc.vector.memset(ia[:, :], 0.0)
        for ms in range(4):
            for lt in range(4):
                nc.tensor.matmul(
                    out=ia[0:LT[ms], ms:ms + 1],
                    lhsT=att[0:LT[lt], lt, LT_OFF[ms]:LT_OFF[ms] + LT[ms]],
                    rhs=ones_bf[0:LT[lt], 0:1],
                    start=(lt == 0), stop=(lt == 3))
        iab = consts.tile([128, 4], DT.float32, tag=f"iatt{b}", name=f"iatt{b}")
        nc.vector.tensor_copy(iab[:, :], ia[:, :])
        iattT[b] = iab

    fts_live = {}

    def ftp_block(s, b):
        ftp = ps_small.tile([128, 4 * F], DT.bfloat16, tag="rot", name="ftp")
        if SIM_INIT:
            nc.vector.memset(ftp[:, :], 0.0)
        for lt in range(4):
            m = LT[lt]
            nc.tensor.transpose(
                ftp[:m, F * lt:F * (lt + 1)],
                feat_sb[(s, b)][0:F, LT_OFF[lt]:LT_OFF[lt] + m],
                ident_bf[0:F, 0:F])
        fts = ft_pool.tile([128, 4 * F], DT.bfloat16, tag=f"fts{s}{b % 4}",
                           name="fts")
        nc.vector.tensor_copy(fts[:, :], ftp[:, :])
        fts_live[(s, b)] = fts

    S_cell = []

    def S_alloc():
        S = ps_g.tile([F, 6 * BLOC], DT.float32, tag="grow", name="S")
        S_cell.append(S)
        return S

    def pool_block(s, b):
        si = 0 if s == "u" else 1
        col = 3 * (BLOC * si + b)
        wa = sm_pool.tile([128, 4, 3], DT.bfloat16, tag="wa", name="wa")
        attsrc = uattT[:, :, b:b + 1] if s == "u" else iattT[b][:, :, None]
        nc.vector.tensor_tensor(out=wa[:, :, :],
                                in0=attsrc.to_broadcast([128, 4, 3]),
                                in1=ck_sb[:, :, :], op=ALU.mult)
        fts = fts_live.pop((s, b))
        S_ps = S_cell[0]
        for lt in range(4):
            m = LT[lt]
            nc.tensor.matmul(out=S_ps[:, col:col + 3],
                             lhsT=fts[:m, F * lt:F * (lt + 1)],
                             rhs=wa[:m, lt, :], start=(lt == 0), stop=(lt == 3))

    # ---------------- wave-scheduled driver ----------------
    # stages pin per-engine order; ACT sees contiguous same-table blocks.
    with tc.tile_wait_until(1):
        gathers_for(0)
        gate_quad(0)
        gathers_for(1)
        gate_quad(1)
        gathers_for(2)
        gate_quad(2)
        gathers_for(3)
        gate_quad(3)
        for b in range(BLOC):
            conv_block("u", b)
            conv_block("i", b)
            sq_mm(b)
            if b < 4:
                ln_block(b)
    with tc.tile_wait_until(2):
        for b in range(4):
            sig_block(b)
    with tc.tile_wait_until(3):
        for b in range(4, BLOC):
            ln_block(b)
    with tc.tile_wait_until(4):
        for b in range(4, BLOC):
            sig_block(b)
    with tc.tile_wait_until(1):
        S_ps = S_alloc()
        for b in range(BLOC):
            iatt_block(b)
            ftp_block("u", b), ftp_block("i", b)
            pool_block("u", b), pool_block("i", b)
    with tc.tile_wait_until(4):
        dbg_sb = consts.tile([128, 512], DT.float32, tag="dbgs", name="dbgs")
        nc.vector.tensor_copy(dbg_sb[0:102, 0:500], feat_sb[("u", 0)][0:102, 0:500])
        nc.sync.dma_start(out=t["dbg"].ap()[:, 0:512], in_=dbg_sb[:, :])
        dbg2 = consts.tile([128, 512], DT.float32, tag="dbg2", name="dbg2")
        nc.vector.tensor_copy(dbg2[0:102, 0:500], feat_sb[("i", 0)][0:102, 0:500])
        nc.sync.dma_start(out=t["dbg"].ap()[:, 512:1024], in_=dbg2[:, :])
        dbg3 = consts.tile([128, 512], DT.float32, tag="dbg3", name="dbg3")
        nc.vector.tensor_copy(dbg3[:, 0:500], att_dbg_cell[0][:, 0, :])
        nc.sync.dma_start(out=t["dbg"].ap()[:, 1024:1536], in_=dbg3[:, :])
        dbg4 = consts.tile([128, 64], DT.float32, tag="dbg4", name="dbg4")
        nc.vector.tensor_copy(dbg4[:, 0:32], uattT[:, 0:4, 0:8])
        nc.sync.dma_start(out=t["dbg"].ap()[:, 1536:1600], in_=dbg4[:, :])
        dbg5 = consts.tile([128, 48], DT.float32, tag="dbg5", name="dbg5")
        nc.vector.tensor_copy(dbg5[0:101, :], S_sb[:, :])
        nc.sync.dma_start(out=t["dbg"].ap()[:, 1600:1648], in_=dbg5[:, :])

        nc.vector.tensor_copy(S_sb[0:F, :], S_cell[0][:, :])

        am_ps = ps_small.tile([F, 2 * BLOC], DT.float32, tag="rot", name="am")
        for si, s in enumerate("ui"):
            for b in range(BLOC):
                for k in range(3):
                    nc.tensor.matmul(
                        out=am_ps[:, BLOC * si + b:BLOC * si + b + 1],
                        lhsT=wabs_sb[s][:, k, :],
                        rhs=S_sb[:, 3 * (BLOC * si + b) + k:3 * (BLOC * si + b) + k + 1],
                        start=(k == 0), stop=(k == 2))
        am_sb = sm_pool.tile([F, 2 * BLOC], DT.bfloat16, tag="am_sb", name="am_sb")
        nc.vector.tensor_copy(am_sb[:, :], am_ps[:, :])

        for si, (s, oname) in enumerate((("u", "out_use"), ("i", "out_item"))):
            fc_ps = ps_small.tile([ID, BLOC], DT.float32, tag="rot", name="fc")
            nc.tensor.matmul(out=fc_ps[:, :], lhsT=wfc_sb[s][:, :],
                             rhs=am_sb[:, BLOC * si:BLOC * (si + 1)],
                             start=True, stop=True)
            fcr = sm_pool.tile([ID, BLOC], DT.float32, tag="fcr", name="fcr")
            nc.scalar.activation(fcr[:, :], fc_ps[:, :], AF.Relu,
                                 bias=bfc_sb[s][:, :])
            fct = ps_small.tile([BLOC, ID], DT.float32, tag="rot", name="fct")
            nc.tensor.transpose(fct[:, :], fcr[:, :], ident_f32[:ID, :ID])
            osb = sm_pool.tile([BLOC, 2 * ID], DT.float32, tag=f"osb{s}", name=f"osb{s}")
            nc.vector.tensor_copy(osb[:, 0:ID], fct[:, :])
            nc.sync.dma_start(out=osb[:, ID:2 * ID], in_=t[f"idrows_{s}"].ap())
            nc.sync.dma_start(out=t[oname].ap(), in_=osb[:, :])

    ctx.close()


# ======================= host side =======================

_PROG = None


def _get_prog():
    global _PROG
    if _PROG is None:
        _PROG = build_program()
    return _PROG


def _bf16_table(tab):
    out = np.zeros((V + 1, DPAD), dtype=BF16)
    out[:V, :D] = np.asarray(tab, dtype=np.float32)
    if os.environ.get("DAML_NOBIAS"):
        out[PADROW, :] = 0  # debug-mode pad row within int16 range
    return out


def _gather_idx(doc):
    """doc: (BLOC, L) ids -> (128, 32*NG) int16 biased index tile"""
    stream = np.full((NG, GW), PADROW, dtype=np.int64)
    for b in range(BLOC):
        stream[b, 1:1 + L] = doc[b]
    biased = (stream - BIAS).astype(np.int16)
    arr = np.zeros((128, 32 * NG), dtype=np.int16)
    for g in range(NG):
        blk = biased[g].reshape(32, 16).T  # idx i -> [i%16, i//16]
        for r in range(8):
            arr[16 * r:16 * (r + 1), 32 * g:32 * (g + 1)] = blk
    return arr


def _window_counts():
    c = np.zeros((3, L), dtype=np.float64)
    for k in range(3):
        for lp in range(k, k + L - 2):
            for d2 in (-1, 0, 1):
                ll = lp + d2
                if 0 <= ll < L:
                    c[k, ll] += 1
    return c


def _prep_weights(inp):
    w = {}
    w3 = np.zeros((DPAD, 3), dtype=np.float32)
    w3[:D, :] = np.asarray(inp["word_cnn_w"][0, 0]).astype(np.float32).T
    w["w3"] = np.ascontiguousarray(w3.reshape(NCH, 128, 3).transpose(1, 0, 2)).astype(BF16)
    w["wcb"] = np.asarray(inp["word_cnn_b"]).astype(np.float32).reshape(1, 1)

    for s, key in (("u", "user"), ("i", "item")):
        dw = np.asarray(inp[f"{key}_doc_cnn_w"]).astype(np.float32)  # (F,1,3,D)
        arr = np.zeros((128, NCH, 3, 102), dtype=BF16)
        for tp in range(3):
            pad = np.zeros((DPAD, F), dtype=np.float32)
            pad[:D] = dw[:, 0, tp, :].T
            arr[:, :, tp, 0:F] = pad.reshape(NCH, 128, F).transpose(1, 0, 2)
        w[f"wd_{s}"] = arr
        dcb = np.asarray(inp[f"{key}_doc_cnn_b"]).astype(np.float32)
        dcb_pad = np.zeros((102, 1), dtype=np.float32)
        dcb_pad[0:100, 0] = dcb * (-2.0 if s == "u" else 1.0)
        w[f"dcb_{s}"] = dcb_pad

        aw = np.asarray(inp[f"{key}_abs_cnn_w"]).astype(np.float32)  # (F,1,3,F)
        ab = np.asarray(inp[f"{key}_abs_cnn_b"]).astype(np.float32)
        scale = (1.0 / (L - 2)) * (-0.5 if s == "u" else 1.0)
        warr = np.zeros((101, 3, F), dtype=BF16)
        for k in range(3):
            warr[:F, k, :] = (aw[:, 0, k, :] * scale).T
        warr[F, 0, :] = ab
        w[f"wabs_{s}"] = warr

        w[f"wfc_{s}"] = np.asarray(inp[f"{key}_fc_w"]).astype(np.float32).T.astype(BF16)
        w[f"bfc_{s}"] = np.asarray(inp[f"{key}_fc_b"]).astype(np.float32).reshape(ID, 1)

    cw = _window_counts()
    ckt = np.zeros((128, 4, 3), dtype=BF16)
    for lt in range(4):
        m = LT[lt]
        ckt[:m, lt, :] = cw[:, LT_OFF[lt]:LT_OFF[lt] + m].T
    w["ck"] = ckt
    return w


def prepare_in_maps(inputs):
    w = _prep_weights(inputs)
    tab_u = _bf16_table(inputs["user_word_emb"])
    tab_i = _bf16_table(inputs["item_word_emb"])
    user_doc = np.asarray(inputs["user_doc"]).astype(np.int64)
    item_doc = np.asarray(inputs["item_doc"]).astype(np.int64)
    uids = np.asarray(inputs["uids"]).astype(np.int64)
    iids = np.asarray(inputs["iids"]).astype(np.int64)
    uid_emb = np.asarray(inputs["uid_emb"]).astype(np.float32)
    iid_emb = np.asarray(inputs["iid_emb"]).astype(np.float32)

    in_maps = []
    for c in range(NCORE):
        sl = slice(BLOC * c, BLOC * (c + 1))
        in_maps.append({
            "tab_u": tab_u, "tab_i": tab_i,
            "idx_u": _gather_idx(user_doc[sl]),
            "idx_i": _gather_idx(item_doc[sl]),
            "w3": w["w3"], "wcb": w["wcb"], "ck": w["ck"],
            "onesd": np.ones((1, 512), dtype=BF16),
            "wd_u": w["wd_u"], "wd_i": w["wd_i"],
            "dcb_u": w["dcb_u"], "dcb_i": w["dcb_i"],
            "wabs_u": w["wabs_u"], "wabs_i": w["wabs_i"],
            "wfc_u": w["wfc_u"], "wfc_i": w["wfc_i"],
            "bfc_u": w["bfc_u"], "bfc_i": w["bfc_i"],
            # crossed on purpose: use_fea carries iid_emb, item_fea uid_emb
            "idrows_u": iid_emb[iids[sl]].astype(np.float32),
            "idrows_i": uid_emb[uids[sl]].astype(np.float32),
        })
    return in_maps


def assemble_outputs(res):
    use = np.concatenate([np.asarray(res.results[c]["out_use"]) for c in range(NCORE)])
    item = np.concatenate([np.asarray(res.results[c]["out_item"]) for c in range(NCORE)])
    return (use.reshape(B, 2, ID).astype(np.float32),
            item.reshape(B, 2, ID).astype(np.float32))


def kernel(**inputs):
    nc = _get_prog()
    in_maps = prepare_in_maps(inputs)
    res = bass_utils.run_bass_kernel_spmd(nc, in_maps, core_ids=list(range(NCORE)))
    return assemble_outputs(res)


# revision 42
# speedup vs baseline: 1.6297x; 1.1080x over previous
"""DAML dense_cnn Trainium2 Bass kernel (v2).

Data-parallel over batch: B=64 -> 8 NeuronCores x 8 batches each.

Per-core pipeline (per side u/i), restructured from v1 for engine balance:
  1. 16 dma_gathers (transpose, bf16, 512 idx each) pull e^T = emb[doc]^T
     into SBUF as (128 dpart, 3 chunks, 512 tok-cols) per batch-side,
     spread over 4 SWDGE queues. int16 index range beaten by biasing the
     table base by 32768 rows (ucode sign-extends). Pad positions gather a
     host-appended zero row.
  2. Gates in column form: per (side,batch) 36 tiny N=1 matmuls (tap
     shift on the lhsT eT slice) accumulate gate[l] on partitions, one
     column per 128-token block; Sigmoid in column form; 4 aligned PE
     transposes -> gate row; gpsimd partition_broadcast; DVE gating mult
     on eT.  PE stream time ~0 vs v1's 144 N=500 matmuls (~20us saved).
  3. conv: 9 matmuls (3 taps x 3 chunks) accumulate feat psum rows 2..101;
     aug rows computed in-psum (row0/1 = ones | sum_f feat^2 via
     K=100 ones-matmul writing psum row directly - no DRAM bounce).
     feat -> sbuf (102, 512) bf16, cols 500:512 zeroed so all sq matmuls
     run M=128.
  4. sq einsum K=102 now includes BOTH norm terms (aug rows), so Ln needs
     no bias -> Ln merged to (128,2,500) ops.  att = sigmoid(-0.5*ln(sq)),
     accum_out gives user row-sums free; item col-sums via N=1
     ones-matmuls (free on PE).
  5. ACT table thrash killed by wave scheduling (tc.tile_wait_until):
     [sig gates x4][Ln b0-3][sig b0-3][Ln b4-7][sig b4-7][relu] = 5 loads.
  6. Pooling: PE transposes of feat + S_k matmuls, abs-conv contraction
     (bias via aug row), fc matmul, ACT relu, PE transpose, id-emb
     indirect gather, DMA out.
"""
import os
import numpy as np
import ml_dtypes

import concourse.bass as bass
import concourse.bacc as bacc
import concourse.tile as tile
from concourse import mybir
from concourse import bass_utils

BF16 = ml_dtypes.bfloat16
DT = mybir.dt
AF = mybir.ActivationFunctionType
ALU = mybir.AluOpType

B, L, V, D, F, ID = 64, 500, 50000, 300, 100, 32
NCORE = 8
BLOC = B // NCORE            # batches per core
DPAD = 384                   # D padded to 3*128
NCH = 3
PADROW = 32760 if os.environ.get("DAML_NOBIAS") else V   # zero row
BIAS = 0 if os.environ.get("DAML_NOBIAS") else 32768   # int16 index bias
GW = 512                     # tokens per gather group (1 batch)
NG = BLOC                    # gather groups per side
LT = [128, 128, 128, 116]
LT_OFF = [0, 128, 256, 384]
GB = [128, 128, 128, 116]    # gate l-blocks (gate cols 0..499)
SIM_INIT = os.environ.get("DAML_SIM_INIT") == "1"   # CoreSim-only psum memsets
LOOP = int(os.environ.get("DAML_LOOP", "1"))


def build_program():
    nc = bacc.Bacc("TRN2", target_bir_lowering=False, debug=False,
                   num_devices=NCORE, num_swdge_queues=4)
    t = {}

    def din(name, shape, dt):
        t[name] = nc.dram_tensor(name, shape, dt, kind="ExternalInput")

    for s in "ui":
        din(f"tab_{s}", (V + 1, DPAD), DT.bfloat16)
        din(f"idx_{s}", (128, 32 * NG), DT.int16)
        din(f"wd_{s}", (128, NCH, 3, 102), DT.bfloat16)
        din(f"dcb_{s}", (102, 1), DT.float32)
        din(f"wabs_{s}", (101, 3, F), DT.bfloat16)
        din(f"wfc_{s}", (F, ID), DT.bfloat16)
        din(f"bfc_{s}", (ID, 1), DT.float32)
        din(f"idrows_{s}", (BLOC, ID), DT.float32)
    din("w3", (128, NCH, 3), DT.bfloat16)
    din("wcb", (1, 1), DT.float32)
    din("ck", (128, 4, 3), DT.bfloat16)
    din("onesd", (1, 512), DT.bfloat16)
    t["dbg"] = nc.dram_tensor("dbg", (128, 4096), DT.float32,
                              kind="ExternalOutput")
    t["out_use"] = nc.dram_tensor("out_use", (BLOC, 2 * ID), DT.float32,
                                  kind="ExternalOutput")
    t["out_item"] = nc.dram_tensor("out_item", (BLOC, 2 * ID), DT.float32,
                                   kind="ExternalOutput")

    with tile.TileContext(nc) as tc:
        _emit(nc, tc, t)

    nc.compile()
    return nc


def _emit(nc, tc, t):
    from contextlib import ExitStack
    from concourse.masks import make_identity
    ctx = ExitStack()

    consts = ctx.enter_context(tc.tile_pool(name="consts", bufs=1))
    et_pool = ctx.enter_context(tc.tile_pool(name="et", bufs=1))
    feat_pool = ctx.enter_context(tc.tile_pool(name="feat", bufs=1))
    gsig_pool = ctx.enter_context(tc.tile_pool(name="gsig", bufs=2))
    lnt_pool = ctx.enter_context(tc.tile_pool(name="lnt", bufs=16))
    att_pool = ctx.enter_context(tc.tile_pool(name="att", bufs=1))
    sm_pool = ctx.enter_context(tc.tile_pool(name="sm", bufs=4))
    gbc_pool = ctx.enter_context(tc.tile_pool(name="gbc", bufs=4))
    u2_pool = ctx.enter_context(tc.tile_pool(name="u2", bufs=2))
    ft_pool = ctx.enter_context(tc.tile_pool(name="ft", bufs=1))

    ps_sq = ctx.enter_context(tc.tile_pool(name="ps_sq", bufs=1, space="PSUM"))
    ps_fps = ctx.enter_context(tc.tile_pool(name="ps_fps", bufs=2, space="PSUM"))
    ps_g = ctx.enter_context(tc.tile_pool(name="ps_g", bufs=1, space="PSUM"))
    ps_small = ctx.enter_context(tc.tile_pool(name="ps_small", bufs=1, space="PSUM"))

    # ---------------- constants / weights ----------------
    idx_sb, wd_sb, wabs_sb, wfc_sb, bfc_sb, dcb_sb, idid_sb = ({} for _ in range(7))
    for s in "ui":
        idx_sb[s] = consts.tile([128, 32 * NG], DT.int16, tag=f"idx{s}", name=f"idx{s}")
        nc.sync.dma_start(out=idx_sb[s][:], in_=t[f"idx_{s}"].ap())
    w3_sb = consts.tile([128, NCH, 3], DT.bfloat16, tag="w3", name="w3")
    nc.sync.dma_start(out=w3_sb[:], in_=t["w3"].ap())
    wcb_sb = consts.tile([1, 1], DT.float32, tag="wcb", name="wcb")
    nc.sync.dma_start(out=wcb_sb[:], in_=t["wcb"].ap())
    wcb_bc = consts.tile([128, 1], DT.float32, tag="wcbb", name="wcbb")
    nc.gpsimd.partition_broadcast(wcb_bc[:, :], wcb_sb[:, :])
    for s in "ui":
        wd_sb[s] = consts.tile([128, NCH, 3, 102], DT.bfloat16, tag=f"wd{s}", name=f"wd{s}")
        nc.sync.dma_start(out=wd_sb[s][:], in_=t[f"wd_{s}"].ap())
        dcb_sb[s] = consts.tile([102, 1], DT.float32, tag=f"dcb{s}", name=f"dcb{s}")
        nc.sync.dma_start(out=dcb_sb[s][:], in_=t[f"dcb_{s}"].ap())
        wabs_sb[s] = consts.tile([101, 3, F], DT.bfloat16, tag=f"wabs{s}", name=f"wabs{s}")
        nc.sync.dma_start(out=wabs_sb[s][:], in_=t[f"wabs_{s}"].ap())
        wfc_sb[s] = consts.tile([F, ID], DT.bfloat16, tag=f"wfc{s}", name=f"wfc{s}")
        nc.sync.dma_start(out=wfc_sb[s][:], in_=t[f"wfc_{s}"].ap())
        bfc_sb[s] = consts.tile([ID, 1], DT.float32, tag=f"bfc{s}", name=f"bfc{s}")
        nc.sync.dma_start(out=bfc_sb[s][:], in_=t[f"bfc_{s}"].ap())

    ck_sb = consts.tile([128, 4, 3], DT.bfloat16, tag="ck", name="ck")
    nc.sync.dma_start(out=ck_sb[:], in_=t["ck"].ap())

    ones_bf = consts.tile([128, 1], DT.bfloat16, tag="ones", name="ones")
    nc.vector.memset(ones_bf[:], 1.0)
    ident_bf = consts.tile([128, 128], DT.bfloat16, tag="identb", name="identb")
    make_identity(nc, ident_bf[:])
    ident_f32 = consts.tile([ID, ID], DT.float32, tag="identf", name="identf")
    make_identity(nc, ident_f32[:])

    # norm-row helper consts (all memset-built):
    # u psum row100 <- -0.5*ones (drain x-2 -> 1), row101 <- -0.125*sum u2
    # i psum row100 <- sum u2 (v2s), row101 <- ones
    onesrow = consts.tile([1, 512], DT.bfloat16, tag="onesrow", name="onesrow")
    nc.vector.memset(onesrow[:, :], 1.0)
    # norm matmuls target psum rows 64..101 (base-64 aligned); cols 36/37
    # of the M=38 lhsT map to psum rows 100/101.
    w6u = consts.tile([100, 38], DT.bfloat16, tag="w6u", name="w6u")
    nc.vector.memset(w6u[:, :], 0.0)
    nc.vector.memset(w6u[:, 37:38], -0.125)
    o6u = consts.tile([1, 38], DT.bfloat16, tag="o6u", name="o6u")
    nc.vector.memset(o6u[:, :], 0.0)
    nc.vector.memset(o6u[:, 36:37], -0.5)
    w6i = consts.tile([100, 38], DT.bfloat16, tag="w6i", name="w6i")
    nc.vector.memset(w6i[:, :], 0.0)
    nc.vector.memset(w6i[:, 36:37], 1.0)
    o6i = consts.tile([1, 38], DT.bfloat16, tag="o6i", name="o6i")
    nc.vector.memset(o6i[:, :], 0.0)
    nc.vector.memset(o6i[:, 37:38], 1.0)

    onesP = consts.tile([128, 500], DT.bfloat16, tag="onesP", name="onesP")
    nc.vector.memset(onesP[:, :], 1.0)

    S_sb = consts.tile([101, 6 * BLOC], DT.bfloat16, tag="Ssb", name="Ssb")
    nc.sync.dma_start(out=S_sb[F:F + 1, :], in_=t["onesd"].ap()[:, 0:6 * BLOC])

    uattT = consts.tile([128, 4, BLOC], DT.float32, tag="uatt", name="uatt")
    iattT = {}

    # feat tiles: rows 0/1 aug, 2..101 feat; cols 500:512 zero (M=128 sq)
    feat_sb = {}
    for s in "ui":
        for b in range(BLOC):
            fsb = feat_pool.tile([102, 512], DT.bfloat16, tag=f"feat{s}{b}",
                                 name=f"feat{s}{b}")
            feat_sb[(s, b)] = fsb
            nc.vector.memset(fsb[:, 500:512], 0.0)
            if s == "u":
                # pad-col lhsT trick: row0 (ones aug row) = 1 at pad cols so
                # the M=128 sq matmuls give sq = v2s > 0 on pad rows
                # (Ln(0) = -inf/NaN otherwise)
                nc.vector.memset(fsb[0:1, 500:512], 1.0)

    if LOOP > 1:
        ctx.enter_context(tc.For_i(0, LOOP, 1))

    # ---------------- gathers (one per batch-side) ----------------
    eT = {}
    _gidx = [0]

    def gathers_for(q):
        for g in (2 * q, 2 * q + 1):
            for si, s in enumerate("ui"):
                eT[(s, g)] = et_pool.tile([128, NCH, GW], DT.bfloat16,
                                          tag=f"eT{s}{g}", name=f"eT{s}{g}")
                nc.gpsimd.dma_gather(
                    out_ap=eT[(s, g)][:],
                    in_ap=t[f"tab_{s}"].ap()[BIAS:, :],
                    idxs_ap=idx_sb[s][:, 32 * g:32 * (g + 1)],
                    num_idxs=GW, num_idxs_reg=GW,
                    elem_size=DPAD, transpose=True,
                    queue_num=0,
                )
                _gidx[0] += 1

    # ---------------- building blocks ----------------
    gate_state = {}

    def gate_quad(q):
        # gates for (u,2q),(i,2q),(u,2q+1),(i,2q+1).  Column form: 36 tiny
        # N=1 matmuls accumulate Sum_{tap,chunk} w3*e[l+tap] into oT[l, B]
        # (tap shift applied on the lhsT slice, so no cross-partition
        # combine is needed); sigmoid in column form; 4 aligned PE
        # transposes -> gate row; broadcast; gate eT in place.
        quad = [("u", 2 * q), ("i", 2 * q), ("u", 2 * q + 1), ("i", 2 * q + 1)]
        oT = ps_small.tile([128, 16], DT.float32, tag="rot", name="oT")
        if SIM_INIT:
            # rows GB[3]..127 of the Bb=3 columns are never matmul-written;
            # harmless junk on HW (excluded downstream), but CoreSim's
            # uninit tracker needs them defined
            nc.vector.memset(oT[:, :], 0.0)
        for qi, (s, g) in enumerate(quad):
            for Bb in range(4):
                m = GB[Bb]
                k = 0
                for tp in range(3):
                    for c in range(NCH):
                        nc.tensor.matmul(
                            out=oT[0:m, 4 * qi + Bb:4 * qi + Bb + 1],
                            lhsT=eT[(s, g)][:, c, 128 * Bb + tp:128 * Bb + tp + m],
                            rhs=w3_sb[:, c, tp:tp + 1],
                            start=(k == 0), stop=(k == 8))
                        k += 1
        gate_state[q] = (quad, oT)
        quad, oT = gate_state.pop(q)
        gcol = sm_pool.tile([128, 16], DT.bfloat16, tag="gcol", name="gcol")
        nc.scalar.activation(gcol[:, :], oT[:, :], AF.Sigmoid,
                             bias=wcb_bc[:, :])
        g_row = ps_g.tile([65, 1024], DT.bfloat16, tag="grow", name="grow")
        if SIM_INIT:
            nc.vector.memset(g_row[:, :], 0.0)
        for qi, (s, g) in enumerate(quad):
            for Bb in range(4):
                m = GB[Bb]
                p0, c0 = (32 * qi, 0) if qi < 3 else (0, 512)
                nc.tensor.matmul(out=g_row[p0:p0 + 1, c0 + 128 * Bb:c0 + 128 * Bb + m],
                                 lhsT=gcol[0:m, 4 * qi + Bb:4 * qi + Bb + 1],
                                 rhs=ident_bf[0:m, 0:m],
                                 is_transpose=True, start=True, stop=True)
        g4s = gsig_pool.tile([65, 1024], DT.bfloat16, tag="gs", name="gs")
        nc.scalar.activation(g4s[:, :], g_row[:, :], AF.Copy)
        if q == 0:
            dbg_g = consts.tile([65, 1024], DT.float32, tag="dbg_g", name="dbg_g")
            nc.vector.tensor_copy(dbg_g[:, :], g4s[:, :])
            nc.sync.dma_start(out=t["dbg"].ap()[0:65, 2048:3072], in_=dbg_g[:, :])
        for qi, (s, g) in enumerate(quad):
            gb = gbc_pool.tile([128, 500], DT.bfloat16, tag=f"gb{qi}", name="gb")
            p0, c0 = (32 * qi, 0) if qi < 3 else (0, 512)
            nc.gpsimd.partition_broadcast(gb[:, :], g4s[p0:p0 + 1, c0:c0 + 500])
            nc.vector.tensor_tensor(out=eT[(s, g)][:, :, 1:501],
                                    in0=eT[(s, g)][:, :, 1:501],
                                    in1=gb[:, None, :].to_broadcast([128, NCH, 500]),
                                    op=ALU.mult)
            if s == "i" and g == 0:
                dbg_e = consts.tile([128, 512], DT.float32, tag="dbg_e", name="dbg_e")
                nc.vector.tensor_copy(dbg_e[:, :], eT[(s, g)][:, 0, :])
                nc.sync.dma_start(out=t["dbg"].ap()[:, 3072:3584], in_=dbg_e[:, :])

    def conv_block(s, b):
        # feat rows 0..99; aug rows 100/101 built in-psum by two extra
        # accumulating matmuls (zero lhsT cols keep feat rows intact):
        #   u: row100 = ones (post x-2), row101 = ||u_l||^2
        #   i: row100 = ||v_m||^2,       row101 = ones
        fps = ps_fps.tile([102, 512], DT.float32, tag="fps", name="fps")
        k = 0
        for tp in range(3):
            for c in range(NCH):
                nc.tensor.matmul(out=fps[0:102, 0:500],
                                 lhsT=wd_sb[s][:, c, tp, :],
                                 rhs=eT[(s, b)][:, c, tp:tp + 500],
                                 start=(k == 0), stop=False,
                                 skip_group_check=True)
                k += 1
        fsb = feat_sb[(s, b)]
        scl = -2.0 if s == "u" else 1.0
        if False:
            nc.scalar.activation(fsb[0:100, 0:500], fps[0:100, 0:500],
                                 AF.Identity, scale=scl,
                                 bias=dcb_sb[s][0:100, :])
        else:
            nc.vector.tensor_scalar(out=fsb[0:100, 0:500], in0=fps[0:100, 0:500],
                                    scalar1=scl, scalar2=dcb_sb[s][0:100, :],
                                    op0=ALU.mult, op1=ALU.add)
        u2t = u2_pool.tile([100, 500], DT.bfloat16, tag="u2", name="u2")
        u2eng = nc.vector if b < 2 else nc.gpsimd
        u2eng.tensor_tensor(out=u2t[0:100, :], in0=fsb[0:100, 0:500],
                            in1=fsb[0:100, 0:500], op=ALU.mult)
        w6, o6 = (w6u, o6u) if s == "u" else (w6i, o6i)
        nc.tensor.matmul(out=fps[64:102, 0:500], lhsT=w6[0:100, :],
                         rhs=u2t[0:100, :], start=False, stop=False,
                         skip_group_check=True)
        nc.tensor.matmul(out=fps[64:102, 0:500], lhsT=o6[0:1, :],
                         rhs=onesrow[0:1, 0:500], start=False, stop=True,
                         skip_group_check=True)
        if False:
            nc.scalar.activation(fsb[64:102, 0:500], fps[64:102, 0:500],
                                 AF.Identity, scale=scl,
                                 bias=dcb_sb[s][64:102, :])
        else:
            nc.vector.tensor_scalar(out=fsb[64:102, 0:500], in0=fps[64:102, 0:500],
                                    scalar1=scl, scalar2=dcb_sb[s][64:102, :],
                                    op0=ALU.mult, op1=ALU.add)

    lnt_live = {}

    def sq_mm(b):
        for half in range(2):
            sq2 = ps_sq.tile([128, 2, 512], DT.float32, tag=f"sq{half}", name="sq")
            for j in range(2):
                lt = 2 * half + j
                nc.tensor.matmul(
                    out=sq2[:, j, 0:500],
                    lhsT=feat_sb[("u", b)][0:102, LT_OFF[lt]:LT_OFF[lt] + 128],
                    rhs=feat_sb[("i", b)][0:102, 0:500],
                    start=True, stop=True)
            lnt_live[(b, half)] = sq2

    def ln_block(b):
        # att = sigmoid(-0.5*ln(sq)) = 1/(1+sqrt(sq)); sq carries both norm
        # terms via the aug rows so Ln needs no bias and merges 2 tiles/op.
        for half in range(2):
            sq2 = lnt_live.pop((b, half))
            lnt = lnt_pool.tile([128, 2, 500], DT.bfloat16, tag="lnt", name="lnt")
            nc.scalar.activation(lnt[:, :, :], sq2[:, :, 0:500], AF.Ln)
            lnt_live[(b, half, "ln")] = lnt

    att_live = {}
    att_dbg_cell = []

    def sig_block(b):
        att = att_pool.tile([128, 4, 500], DT.bfloat16, tag=f"att{b}",
                            name=f"att{b}")
        for half in range(2):
            lnt = lnt_live.pop((b, half, "ln"))
            for j in range(2):
                lt = 2 * half + j
                nc.scalar.activation(att[:, lt, :], lnt[:, j, :],
                                     AF.Sigmoid, scale=-0.5,
                                     accum_out=uattT[:, lt, b:b + 1])
        att_live[b] = att
        if b == 0:
            att_dbg_cell.append(att)

    def iatt_block(b):
        att = att_live.pop(b)
        ia = ps_small.tile([128, 4], DT.float32, tag="rot", name="ia")
        if SIM_INIT:
            nc.vector.memset(ia[:, :], 0.0)
        for ms in range(4):
            for lt in range(4):
                nc.tensor.matmul(
                    out=ia[0:LT[ms], ms:ms + 1],
                    lhsT=att[0:LT[lt], lt, LT_OFF[ms]:LT_OFF[ms] + LT[ms]],
                    rhs=ones_bf[0:LT[lt], 0:1],
                    start=(lt == 0), stop=(lt == 3))
        iab = consts.tile([128, 4], DT.float32, tag=f"iatt{b}", name=f"iatt{b}")
        nc.vector.tensor_copy(iab[:, :], ia[:, :])
        iattT[b] = iab

    fts_live = {}

    def ftp_block(s, b):
        ftp = ps_small.tile([128, 4 * F], DT.bfloat16, tag="rot", name="ftp")
        if SIM_INIT:
            nc.vector.memset(ftp[:, :], 0.0)
        for lt in range(4):
            m = LT[lt]
            nc.tensor.transpose(
                ftp[:m, F * lt:F * (lt + 1)],
                feat_sb[(s, b)][0:F, LT_OFF[lt]:LT_OFF[lt] + m],
                ident_bf[0:F, 0:F])
        fts = ft_pool.tile([128, 4 * F], DT.bfloat16, tag=f"fts{s}{b % 4}",
                           name="fts")
        nc.vector.tensor_copy(fts[:, :], ftp[:, :])
        fts_live[(s, b)] = fts

    S_cell = []

    def S_alloc():
        S = ps_g.tile([F, 6 * BLOC], DT.float32, tag="grow", name="S")
        S_cell.append(S)
        return S

    def pool_block(s, b):
        si = 0 if s == "u" else 1
        col = 3 * (BLOC * si + b)
        wa = sm_pool.tile([128, 4, 3], DT.bfloat16, tag="wa", name="wa")
        attsrc = uattT[:, :, b:b + 1] if s == "u" else iattT[b][:, :, None]
        nc.vector.tensor_tensor(out=wa[:, :, :],
                                in0=attsrc.to_broadcast([128, 4, 3]),
                                in1=ck_sb[:, :, :], op=ALU.mult)
        fts = fts_live.pop((s, b))
        S_ps = S_cell[0]
        for lt in range(4):
            m = LT[lt]
            nc.tensor.matmul(out=S_ps[:, col:col + 3],
                             lhsT=fts[:m, F * lt:F * (lt + 1)],
                             rhs=wa[:m, lt, :], start=(lt == 0), stop=(lt == 3))

    # ---------------- wave-scheduled driver ----------------
    # stages pin per-engine order; ACT sees contiguous same-table blocks.
    with tc.tile_wait_until(1):
        # accum_out READS the accumulator: zero it each iteration
        nc.vector.memset(uattT[:, :, :], 0.0)
        gathers_for(0)
        gate_quad(0)
        gathers_for(1)
        gate_quad(1)
        gathers_for(2)
        gate_quad(2)
        gathers_for(3)
        gate_quad(3)
        for b in range(BLOC):
            conv_block("u", b)
            conv_block("i", b)
            sq_mm(b)
            if b < 4:
                ln_block(b)
    with tc.tile_wait_until(2):
        for b in range(4):
            sig_block(b)
    with tc.tile_wait_until(3):
        for b in range(4, BLOC):
            ln_block(b)
    with tc.tile_wait_until(4):
        for b in range(4, BLOC):
            sig_block(b)
    with tc.tile_wait_until(1):
        S_ps = S_alloc()
        for b in range(BLOC):
            iatt_block(b)
            ftp_block("u", b), ftp_block("i", b)
            pool_block("u", b), pool_block("i", b)
    with tc.tile_wait_until(4):
        dbg_sb = consts.tile([128, 512], DT.float32, tag="dbgs", name="dbgs")
        nc.vector.tensor_copy(dbg_sb[0:102, 0:500], feat_sb[("u", 0)][0:102, 0:500])
        nc.sync.dma_start(out=t["dbg"].ap()[:, 0:512], in_=dbg_sb[:, :])
        dbg2 = consts.tile([128, 512], DT.float32, tag="dbg2", name="dbg2")
        nc.vector.tensor_copy(dbg2[0:102, 0:500], feat_sb[("i", 0)][0:102, 0:500])
        nc.sync.dma_start(out=t["dbg"].ap()[:, 512:1024], in_=dbg2[:, :])
        dbg3 = consts.tile([128, 512], DT.float32, tag="dbg3", name="dbg3")
        nc.vector.tensor_copy(dbg3[:, 0:500], att_dbg_cell[0][:, 0, :])
        nc.sync.dma_start(out=t["dbg"].ap()[:, 1024:1536], in_=dbg3[:, :])
        dbg4 = consts.tile([128, 64], DT.float32, tag="dbg4", name="dbg4")
        nc.vector.tensor_copy(dbg4[:, 0:32], uattT[:, 0:4, 0:8])
        nc.sync.dma_start(out=t["dbg"].ap()[:, 1536:1600], in_=dbg4[:, :])
        dbg5 = consts.tile([128, 48], DT.float32, tag="dbg5", name="dbg5")
        nc.vector.tensor_copy(dbg5[0:101, :], S_sb[:, :])
        nc.sync.dma_start(out=t["dbg"].ap()[:, 1600:1648], in_=dbg5[:, :])

        nc.vector.tensor_copy(S_sb[0:F, :], S_cell[0][:, :])

        am_ps = ps_small.tile([F, 2 * BLOC], DT.float32, tag="rot", name="am")
        for si, s in enumerate("ui"):
            for b in range(BLOC):
                for k in range(3):
                    nc.tensor.matmul(
                        out=am_ps[:, BLOC * si + b:BLOC * si + b + 1],
                        lhsT=wabs_sb[s][:, k, :],
                        rhs=S_sb[:, 3 * (BLOC * si + b) + k:3 * (BLOC * si + b) + k + 1],
                        start=(k == 0), stop=(k == 2))
        am_sb = sm_pool.tile([F, 2 * BLOC], DT.bfloat16, tag="am_sb", name="am_sb")
        nc.vector.tensor_copy(am_sb[:, :], am_ps[:, :])

        for si, (s, oname) in enumerate((("u", "out_use"), ("i", "out_item"))):
            fc_ps = ps_small.tile([ID, BLOC], DT.float32, tag="rot", name="fc")
            nc.tensor.matmul(out=fc_ps[:, :], lhsT=wfc_sb[s][:, :],
                             rhs=am_sb[:, BLOC * si:BLOC * (si + 1)],
                             start=True, stop=True)
            fcr = sm_pool.tile([ID, BLOC], DT.float32, tag="fcr", name="fcr")
            nc.scalar.activation(fcr[:, :], fc_ps[:, :], AF.Relu,
                                 bias=bfc_sb[s][:, :])
            fct = ps_small.tile([BLOC, ID], DT.float32, tag="rot", name="fct")
            nc.tensor.transpose(fct[:, :], fcr[:, :], ident_f32[:ID, :ID])
            osb = sm_pool.tile([BLOC, 2 * ID], DT.float32, tag=f"osb{s}", name=f"osb{s}")
            nc.vector.tensor_copy(osb[:, 0:ID], fct[:, :])
            nc.sync.dma_start(out=osb[:, ID:2 * ID], in_=t[f"idrows_{s}"].ap())
            nc.sync.dma_start(out=t[oname].ap(), in_=osb[:, :])

    ctx.close()


# ======================= host side =======================

_PROG = None


def _get_prog():
    global _PROG
    if _PROG is None:
        _PROG = build_program()
    return _PROG


def _bf16_table(tab):
    out = np.zeros((V + 1, DPAD), dtype=BF16)
    out[:V, :D] = np.asarray(tab, dtype=np.float32)
    if os.environ.get("DAML_NOBIAS"):
        out[PADROW, :] = 0  # debug-mode pad row within int16 range
    return out


def _gather_idx(doc):
    """doc: (BLOC, L) ids -> (128, 32*NG) int16 biased index tile"""
    stream = np.full((NG, GW), PADROW, dtype=np.int64)
    for b in range(BLOC):
        stream[b, 1:1 + L] = doc[b]
    biased = (stream - BIAS).astype(np.int16)
    arr = np.zeros((128, 32 * NG), dtype=np.int16)
    for g in range(NG):
        blk = biased[g].reshape(32, 16).T  # idx i -> [i%16, i//16]
        for r in range(8):
            arr[16 * r:16 * (r + 1), 32 * g:32 * (g + 1)] = blk
    return arr


def _window_counts():
    c = np.zeros((3, L), dtype=np.float64)
    for k in range(3):
        for lp in range(k, k + L - 2):
            for d2 in (-1, 0, 1):
                ll = lp + d2
                if 0 <= ll < L:
                    c[k, ll] += 1
    return c


def _prep_weights(inp):
    w = {}
    w3 = np.zeros((DPAD, 3), dtype=np.float32)
    w3[:D, :] = np.asarray(inp["word_cnn_w"][0, 0]).astype(np.float32).T
    w["w3"] = np.ascontiguousarray(w3.reshape(NCH, 128, 3).transpose(1, 0, 2)).astype(BF16)
    w["wcb"] = np.asarray(inp["word_cnn_b"]).astype(np.float32).reshape(1, 1)

    for s, key in (("u", "user"), ("i", "item")):
        dw = np.asarray(inp[f"{key}_doc_cnn_w"]).astype(np.float32)  # (F,1,3,D)
        arr = np.zeros((128, NCH, 3, 102), dtype=BF16)
        for tp in range(3):
            pad = np.zeros((DPAD, F), dtype=np.float32)
            pad[:D] = dw[:, 0, tp, :].T
            arr[:, :, tp, 0:F] = pad.reshape(NCH, 128, F).transpose(1, 0, 2)
        w[f"wd_{s}"] = arr
        dcb = np.asarray(inp[f"{key}_doc_cnn_b"]).astype(np.float32)
        dcb_pad = np.zeros((102, 1), dtype=np.float32)
        dcb_pad[0:100, 0] = dcb * (-2.0 if s == "u" else 1.0)
        w[f"dcb_{s}"] = dcb_pad

        aw = np.asarray(inp[f"{key}_abs_cnn_w"]).astype(np.float32)  # (F,1,3,F)
        ab = np.asarray(inp[f"{key}_abs_cnn_b"]).astype(np.float32)
        scale = (1.0 / (L - 2)) * (-0.5 if s == "u" else 1.0)
        warr = np.zeros((101, 3, F), dtype=BF16)
        for k in range(3):
            warr[:F, k, :] = (aw[:, 0, k, :] * scale).T
        warr[F, 0, :] = ab
        w[f"wabs_{s}"] = warr

        w[f"wfc_{s}"] = np.asarray(inp[f"{key}_fc_w"]).astype(np.float32).T.astype(BF16)
        w[f"bfc_{s}"] = np.asarray(inp[f"{key}_fc_b"]).astype(np.float32).reshape(ID, 1)

    cw = _window_counts()
    ckt = np.zeros((128, 4, 3), dtype=BF16)
    for lt in range(4):
        m = LT[lt]
        ckt[:m, lt, :] = cw[:, LT_OFF[lt]:LT_OFF[lt] + m].T
    w["ck"] = ckt
    return w


def prepare_in_maps(inputs):
    w = _prep_weights(inputs)
    tab_u = _bf16_table(inputs["user_word_emb"])
    tab_i = _bf16_table(inputs["item_word_emb"])
    user_doc = np.asarray(inputs["user_doc"]).astype(np.int64)
    item_doc = np.asarray(inputs["item_doc"]).astype(np.int64)
    uids = np.asarray(inputs["uids"]).astype(np.int64)
    iids = np.asarray(inputs["iids"]).astype(np.int64)
    uid_emb = np.asarray(inputs["uid_emb"]).astype(np.float32)
    iid_emb = np.asarray(inputs["iid_emb"]).astype(np.float32)

    in_maps = []
    for c in range(NCORE):
        sl = slice(BLOC * c, BLOC * (c + 1))
        in_maps.append({
            "tab_u": tab_u, "tab_i": tab_i,
            "idx_u": _gather_idx(user_doc[sl]),
            "idx_i": _gather_idx(item_doc[sl]),
            "w3": w["w3"], "wcb": w["wcb"], "ck": w["ck"],
            "onesd": np.ones((1, 512), dtype=BF16),
            "wd_u": w["wd_u"], "wd_i": w["wd_i"],
            "dcb_u": w["dcb_u"], "dcb_i": w["dcb_i"],
            "wabs_u": w["wabs_u"], "wabs_i": w["wabs_i"],
            "wfc_u": w["wfc_u"], "wfc_i": w["wfc_i"],
            "bfc_u": w["bfc_u"], "bfc_i": w["bfc_i"],
            # crossed on purpose: use_fea carries iid_emb, item_fea uid_emb
            "idrows_u": iid_emb[iids[sl]].astype(np.float32),
            "idrows_i": uid_emb[uids[sl]].astype(np.float32),
        })
    return in_maps


def assemble_outputs(res):
    use = np.concatenate([np.asarray(res.results[c]["out_use"]) for c in range(NCORE)])
    item = np.concatenate([np.asarray(res.results[c]["out_item"]) for c in range(NCORE)])
    return (use.reshape(B, 2, ID).astype(np.float32),
            item.reshape(B, 2, ID).astype(np.float32))


def kernel(**inputs):
    nc = _get_prog()
    in_maps = prepare_in_maps(inputs)
    res = bass_utils.run_bass_kernel_spmd(nc, in_maps, core_ids=list(range(NCORE)))
    return assemble_outputs(res)
